# revision 15
# baseline (speedup 1.0000x reference)
"""Trainium2 Bass kernel: attention block (QKV + RoPE + ALiBi attention + proj).

Problem shapes: x [2, 2048, 1024], 16 heads x 64 dim, f32 I/O.
Sharding: batch (2) x head-groups (4 heads/core) = 8 cores; AllToAll regroups
per-head outputs into per-query-quarter shards for the output projection.

Dataflow is fully transposed on-chip:
  xT [C, N] --QKV--> qT/kT [D, N] (RoPE'd, d pairwise-interleaved) and v [N, D]
  sT = kT_chunk.T @ qT   [j, i]  -> p = exp(sT) * ebias_table  (no max-sub)
  oT = v_aug.T @ p       [d+1, i] (ones column gives softmax denominators)
  yT = oT[:64] * (1/denom broadcast)  -> A2A -> outT = pwT.T @ yT + b
Host transposes final outT back. All matmuls bf16 with f32 PSUM accumulation.
"""
import sys
if "/opt/trn_rl_repo" not in sys.path:
    sys.path.insert(0, "/opt/trn_rl_repo")

import math
import numpy as np
import ml_dtypes

import concourse.bass as bass
import concourse.mybir as mybir
import concourse.tile as tile
from concourse import bacc
from concourse.bass_utils import run_bass_kernel_spmd

BF = mybir.dt.bfloat16
F32 = mybir.dt.float32
NPBF = ml_dtypes.bfloat16

B, N, C = 2, 2048, 1024
H, D = 16, 64
G = 4                       # heads per core
N_CORES = 8
MAX_BIAS = 8.0
SCALE = D ** -0.5
RG8 = [[0, 1, 2, 3, 4, 5, 6, 7]]

# head dealing (snake by per-head band cost) and per-slot band cutoffs.
# Slot cut must be >= the cut of every head assigned to that slot.
GROUP_HEADS = [[15, 8, 7, 0], [14, 9, 6, 1], [13, 10, 5, 2], [12, 11, 4, 3]]
SLOT_CUT = [1024, 384, 256, 256]

NJ = N // 128               # 16 j-chunks
NI = N // 512               # 4 i-slices
EB_W = 4096                 # ebias table width (offset t = i - j + 2047)

# d-permutation inside each head: [0, 32, 1, 33, ...] so rotate_half becomes
# an even/odd partition swap (StreamShuffle-able within 32-row quadrants).
D_PERM = [x for i in range(32) for x in (i, i + 32)]
SHUF_MASK = [x for i in range(16) for x in (2 * i + 1, 2 * i)]


def _head_cut(slope8):
    c = 26.0 / slope8
    return min(N, 128 * math.ceil((c + 1) / 128) + 128)


def kept_j_chunks(cut, i0):
    return [j for j in range(NJ) if i0 - (j * 128 + 127) <= cut]


def build_program(dbg=False):
    nc = bacc.Bacc("TRN2", target_bir_lowering=False, debug=False,
                   num_devices=N_CORES)
    dbg_outs = {}

    def dbg_tensor(name, shape, dt_=F32):
        dbg_outs[name] = nc.dram_tensor(name, shape, dt_, kind="ExternalOutput")
        return dbg_outs[name]

    xT = nc.dram_tensor("xT", [C, N], BF, kind="ExternalInput")
    wT = nc.dram_tensor("wT", [C, 768], BF, kind="ExternalInput")
    ctq = nc.dram_tensor("ctq", [128, N], F32, kind="ExternalInput")
    stq = nc.dram_tensor("stq", [128, N], F32, kind="ExternalInput")
    ctk = nc.dram_tensor("ctk", [128, N], F32, kind="ExternalInput")
    stk = nc.dram_tensor("stk", [128, N], F32, kind="ExternalInput")
    ebias = nc.dram_tensor("ebias", [G, 128, EB_W], BF, kind="ExternalInput")
    pwT = nc.dram_tensor("pwT", [2 * C, C], BF, kind="ExternalInput")
    pb = nc.dram_tensor("pb", [128, 8], F32, kind="ExternalInput")
    out = nc.dram_tensor("out", [C, 512], F32, kind="ExternalOutput")

    with tile.TileContext(nc) as tc:
        with tc.tile_pool(name="persist", bufs=1) as pp, \
             tc.tile_pool(name="work", bufs=1) as wp, \
             tc.tile_pool(name="psum", bufs=1, space="PSUM") as psp, \
             tc.tile_pool(name="dram", bufs=1, space="DRAM") as dp:

            # ---- persistent tiles (live across phases) ----
            qkp = [pp.tile([128, N], BF, name=f"qk{m}") for m in range(4)]
            vts = [pp.tile([128, G * 65], BF, name=f"vt{j}") for j in range(NJ)]
            yts = [pp.tile([128, N], BF, name=f"yt{i}") for i in range(2)]
            pbt = pp.tile([128, 8], F32, name="pbt")
            nc.sync.dma_start(pbt[:], pb.ap()[:, :])

            with tc.tile_pool(name="ph1", bufs=1) as p1:
                # ---- phase-1 inputs ----
                xts = []
                for ci in range(8):
                    t = p1.tile([128, N], BF, name=f"xt{ci}")
                    nc.sync.dma_start(t[:], xT.ap()[ci * 128:(ci + 1) * 128, :])
                    xts.append(t)
                wts = []
                for ci in range(8):
                    t = p1.tile([128, 768], BF, name=f"wt{ci}")
                    nc.sync.dma_start(t[:], wT.ap()[ci * 128:(ci + 1) * 128, :])
                    wts.append(t)
                tabs = {}
                for nm, dt_ in (("ctq", ctq), ("stq", stq), ("ctk", ctk),
                                ("stk", stk)):
                    t = p1.tile([128, N], F32, name=f"tab_{nm}")
                    nc.sync.dma_start(t[:], dt_.ap()[:, :])
                    tabs[nm] = t

                # ---- QKV (q/k) + RoPE ----
                # wT cols: q01 | q23 | k01 | k23 | v (4x64)
                for m in range(4):
                    ct = tabs["ctq"] if m < 2 else tabs["ctk"]
                    st = tabs["stq"] if m < 2 else tabs["stk"]
                    for isl in range(NI):
                        sl = slice(isl * 512, (isl + 1) * 512)
                        ps = psp.tile([128, 512], F32, name="qkvps",
                                      tag="qkvps", bufs=2)
                        for ci in range(8):
                            nc.tensor.matmul(ps[:],
                                             wts[ci][:, m * 128:(m + 1) * 128],
                                             xts[ci][:, sl],
                                             start=(ci == 0), stop=(ci == 7))
                        rot = wp.tile([128, 512], F32, name="rot", tag="rot",
                                      bufs=2)
                        nc.vector.stream_shuffle(rot[:], ps[:], SHUF_MASK)
                        t1 = wp.tile([128, 512], F32, name="ropet1",
                                     tag="ropet1", bufs=2)
                        nc.vector.tensor_mul(t1[:], rot[:], st[:, sl])
                        t2 = wp.tile([128, 512], F32, name="ropet2",
                                     tag="ropet2", bufs=2)
                        nc.vector.tensor_mul(t2[:], ps[:], ct[:, sl])
                        nc.vector.tensor_add(qkp[m][:, sl], t2[:], t1[:])

                # ---- V (non-transposed, direct) ----
                for j in range(NJ):
                    pv = psp.tile([128, 256], F32, name="vps", tag="vps", bufs=2)
                    for ci in range(8):
                        nc.tensor.matmul(pv[:], xts[ci][:, j * 128:(j + 1) * 128],
                                         wts[ci][:, 512:768],
                                         start=(ci == 0), stop=(ci == 7))
                    vt = vts[j]
                    vt_v = vt[:].rearrange("p (h e) -> p h e", e=65)
                    nc.vector.tensor_copy(vt_v[:, :, 0:64],
                                          pv[:].rearrange("p (h e) -> p h e",
                                                          e=64))
                    nc.vector.memset(vt_v[:, :, 64:65], 1.0)

            # ---- attention per head slot ----
            ebs = []
            ebpool = tc.tile_pool(name="ph2", bufs=1)
            p2 = ebpool.__enter__()
            for s in range(G):
                t = p2.tile([128, EB_W], BF, name=f"eb{s}")
                nc.sync.dma_start(t[:], ebias.ap()[s, :, :])
                ebs.append(t)
            for slot in range(G):
                qh = qkp[slot // 2][(slot % 2) * 64:(slot % 2) * 64 + 64, :]
                kh = qkp[2 + slot // 2][(slot % 2) * 64:(slot % 2) * 64 + 64, :]
                cut = SLOT_CUT[slot]
                ebt = ebs[slot]
                for isl in range(NI):
                    i0 = isl * 512
                    kept = kept_j_chunks(cut, i0)
                    oT = psp.tile([65, 512], F32, name="oT", tag="oT", bufs=2)
                    for idx, j in enumerate(kept):
                        j0 = j * 128
                        s = psp.tile([128, 512], F32, name="s", tag="s", bufs=2)
                        nc.tensor.matmul(s[:], kh[:, j0:j0 + 128],
                                         qh[:, i0:i0 + 512], start=True, stop=True)
                        p = wp.tile([128, 512], BF, name="p", tag="p", bufs=3)
                        nc.scalar.activation(p[:], s[:],
                                             mybir.ActivationFunctionType.Exp)
                        if j0 - i0 < 512:
                            off = i0 - j0 + EB_W // 2 - 1
                            nc.vector.tensor_mul(p[:], p[:], ebt[:, off:off + 512])
                        nc.tensor.matmul(oT[:], vts[j][:, slot * 65:slot * 65 + 65],
                                         p[:], start=(idx == 0),
                                         stop=(idx == len(kept) - 1))
                    den = wp.tile([1, 512], F32, name="den", tag="den", bufs=2)
                    nc.vector.tensor_copy(den[:], oT[64:65, :])
                    rec = wp.tile([1, 512], F32, name="rec", tag="rec", bufs=2)
                    nc.vector.reciprocal_approx_fast(rec[:], den[:])
                    R = wp.tile([64, 512], F32, name="R", tag="R", bufs=2)
                    nc.gpsimd.partition_broadcast(R[:], rec[:])
                    if dbg and slot == 0 and isl == 0:
                        dboT = wp.tile([65, 512], F32, name="dbgoT")
                        nc.vector.tensor_copy(dboT[:], oT[:])
                        t = dbg_tensor("dbg_oT", [65, 512])
                        nc.sync.dma_start(t.ap()[:, :], dboT[:])
                        t2_ = dbg_tensor("dbg_rec", [1, 512])
                        nc.sync.dma_start(t2_.ap()[:, :], rec[:])
                        t3_ = dbg_tensor("dbg_R", [64, 512])
                        nc.sync.dma_start(t3_.ap()[:, :], R[:])
                    yt = yts[slot // 2]
                    r0 = (slot % 2) * 64
                    nc.vector.tensor_mul(yt[r0:r0 + 64, i0:i0 + 512],
                                         oT[0:64, :], R[:])
            ebpool.__exit__(None, None, None)

            if dbg:
                for m in range(4):
                    t = dbg_tensor(f"dbg_qk{m}", [128, N], BF)
                    nc.sync.dma_start(t.ap()[:, :], qkp[m][:])
                t = dbg_tensor("dbg_vt0", [128, G * 65], BF)
                nc.sync.dma_start(t.ap()[:, :], vts[0][:])
                for i in range(2):
                    t = dbg_tensor(f"dbg_yt{i}", [128, N], BF)
                    nc.sync.dma_start(t.ap()[:, :], yts[i][:])

            # ---- AllToAll (8-core): shard j = (my heads, q-quarter j%4) ----
            # out shard i = core i's heads for my quarter; other-batch rows are
            # neutralized by zero rows in pwT.
            a2a_in = dp.tile([2048, 512], BF, name="a2a_in")
            a2a_out = dp.tile([2048, 512], BF, name="a2a_out")
            for shard in range(8):
                q = shard % 4
                for slot in range(G):
                    r = 256 * shard + 64 * slot
                    nc.sync.dma_start(
                        a2a_in[r:r + 64, :],
                        yts[slot // 2][(slot % 2) * 64:(slot % 2) * 64 + 64,
                                       q * 512:(q + 1) * 512])
            nc.gpsimd.collective_compute(
                "AllToAll", mybir.AluOpType.bypass, replica_groups=RG8,
                ins=[a2a_in.opt()], outs=[a2a_out.opt()])

            with tc.tile_pool(name="ph3", bufs=1) as p3:
                pwts = []
                for ci in range(16):
                    t = p3.tile([128, C], BF, name=f"pwt{ci}")
                    nc.sync.dma_start(t[:], pwT.ap()[ci * 128:(ci + 1) * 128, :])
                    pwts.append(t)
                ytf = []
                for ci in range(16):
                    t = p3.tile([128, 512], BF, name=f"ytf{ci}")
                    nc.sync.dma_start(t[:], a2a_out[ci * 128:(ci + 1) * 128, :])
                    ytf.append(t)

                # ---- projection: outT [co, my q-quarter] ----
                for co in range(8):
                    pj = psp.tile([128, 512], F32, name="pj", tag="s", bufs=2)
                    for ci in range(16):
                        nc.tensor.matmul(pj[:],
                                         pwts[ci][:, co * 128:(co + 1) * 128],
                                         ytf[ci][:], start=(ci == 0),
                                         stop=(ci == 15))
                    ot = wp.tile([128, 512], F32, name="ot", tag="ot", bufs=2)
                    nc.scalar.add(ot[:], pj[:], pbt[:, co:co + 1])
                    nc.sync.dma_start(out.ap()[co * 128:(co + 1) * 128, :], ot[:])

    nc.compile()
    return nc


def prep_inputs(x, qkv_w, proj_w, proj_b, slopes):
    """Build the 8 per-core input maps (all host-side numpy)."""
    x = np.asarray(x, np.float32)
    qkv_w = np.asarray(qkv_w, np.float32)
    proj_w = np.asarray(proj_w, np.float32)
    proj_b = np.asarray(proj_b, np.float32)
    slopes = np.asarray(slopes, np.float32)

    # RoPE tables (transposed [d, n], d pairwise-interleaved, x2 head copies)
    inv = 1.0 / (10000.0 ** (np.arange(0, D, 2, dtype=np.float32) / D))
    fr = np.arange(N, dtype=np.float32)[:, None] * inv[None, :]   # [N, 32]
    sin_t, cos_t = np.sin(fr), np.cos(fr)                          # [N, 32]
    ct64 = np.empty((64, N), np.float32)
    st64 = np.empty((64, N), np.float32)
    ct64[0::2] = cos_t.T
    ct64[1::2] = cos_t.T
    st64[0::2] = -sin_t.T      # row for first-half d: -sin
    st64[1::2] = sin_t.T       # row for second-half d: +sin
    ctq = np.vstack([ct64, ct64]) * SCALE
    stq = np.vstack([st64, st64]) * SCALE
    ctk = np.vstack([ct64, ct64])
    stk = np.vstack([st64, st64])

    pos_p = np.arange(128, dtype=np.float64)[:, None]
    t_off = np.arange(EB_W, dtype=np.float64)[None, :] - (EB_W // 2 - 1)
    dmin = np.minimum(pos_p - t_off, 0.0)  # j - i clipped

    in_maps = []
    for core in range(N_CORES):
        b = core // 4
        heads = GROUP_HEADS[core % 4]
        # wT: qkv_w rows for my heads, d-interleaved for q/k, transposed
        rows = []
        for kind in range(2):  # q, k (d-permuted)
            for h in heads:
                base = kind * C + h * D
                rows.extend(base + p for p in D_PERM)
        for h in heads:        # v (natural d order)
            rows.extend(2 * C + h * D + d for d in range(D))
        w_sel = qkv_w[rows, :]                     # [768, 1024]
        wT_c = np.ascontiguousarray(w_sel.T)       # [1024, 768]

        eb_c = np.empty((G, 128, EB_W), np.float32)
        for s, h in enumerate(heads):
            eb_c[s] = np.exp(float(slopes[h]) * MAX_BIAS * dmin)

        # pwT rows (ci) ordered as A2A output: shard i = core i (batch i//4,
        # group i%4), heads in slot order, d natural. Other-batch rows zero.
        pwT_c = np.zeros((2 * C, C), np.float32)
        for src in range(N_CORES):
            if src // 4 != b:
                continue
            for slot, h in enumerate(GROUP_HEADS[src % 4]):
                r = 256 * src + 64 * slot
                pwT_c[r:r + 64, :] = proj_w[:, h * D:(h + 1) * D].T
        pb_c = np.ascontiguousarray(proj_b.reshape(8, 128).T)  # [128, 8]

        in_maps.append({
            "xT": np.ascontiguousarray(x[b].T).astype(NPBF),
            "wT": wT_c.astype(NPBF),
            "ctq": ctq, "stq": stq, "ctk": ctk, "stk": stk,
            "ebias": eb_c.astype(NPBF),
            "pwT": pwT_c.astype(NPBF),
            "pb": pb_c,
        })
    return in_maps


_NC = None


def _get_nc():
    global _NC
    if _NC is None:
        _NC = build_program()
    return _NC


def run(inputs, trace=False):
    nc = _get_nc()
    in_maps = prep_inputs(**inputs)
    res = run_bass_kernel_spmd(nc, in_maps, core_ids=list(range(N_CORES)),
                               trace=trace)
    out = np.empty((B, N, C), np.float32)
    for core in range(N_CORES):
        b, g = core // 4, core % 4
        out[b, g * 512:(g + 1) * 512, :] = res.results[core]["out"].T
    return out, res


def kernel(**inputs) -> np.ndarray:
    out, _ = run(inputs, trace=False)
    return out


# revision 20
# speedup vs baseline: 1.6778x; 1.6778x over previous
"""Trainium2 Bass kernel: attention block (QKV + RoPE + ALiBi attention + proj).

Problem shapes: x [2, 2048, 1024], 16 heads x 64 dim, f32 I/O.
Sharding: batch (2) x head-groups (4 heads/core) = 8 cores; AllToAll regroups
per-head outputs into per-query-quarter shards for the output projection.

Dataflow is fully transposed on-chip:
  xT [C, N] --QKV--> qT/kT [D, N] (RoPE'd, d pairwise-interleaved) and v [N, D]
  sT = kT_chunk.T @ qT   [j, i]  -> p = exp(sT) * ebias_table  (no max-sub)
  oT = v_aug.T @ p       [d+1, i] (ones column gives softmax denominators)
  yT = oT[:64] * (1/denom broadcast)  -> A2A -> outT = pwT.T @ yT + b
Host transposes final outT back. All matmuls bf16 with f32 PSUM accumulation.
"""
import sys
if "/opt/trn_rl_repo" not in sys.path:
    sys.path.insert(0, "/opt/trn_rl_repo")

import math
import numpy as np
import ml_dtypes

import concourse.bass as bass
import concourse.mybir as mybir
import concourse.tile as tile
from concourse import bacc
from concourse.bass_utils import run_bass_kernel_spmd

BF = mybir.dt.bfloat16
F32 = mybir.dt.float32
NPBF = ml_dtypes.bfloat16

B, N, C = 2, 2048, 1024
H, D = 16, 64
G = 4                       # heads per core
N_CORES = 8
MAX_BIAS = 8.0
SCALE = D ** -0.5
RG8 = [[0, 1, 2, 3, 4, 5, 6, 7]]

# head dealing (snake by per-head band cost) and per-slot band cutoffs.
# Slot cut must be >= the cut of every head assigned to that slot.
GROUP_HEADS = [[15, 8, 7, 0], [14, 9, 6, 1], [13, 10, 5, 2], [12, 11, 4, 3]]
SLOT_CUT = [1024, 384, 256, 256]

NJ = N // 128               # 16 j-chunks
NI = N // 512               # 4 i-slices
EB_W = 4096                 # ebias table width (offset t = i - j + 2047)

# d-permutation inside each head: [0, 32, 1, 33, ...] so rotate_half becomes
# an even/odd partition swap (StreamShuffle-able within 32-row quadrants).
D_PERM = [x for i in range(32) for x in (i, i + 32)]
SHUF_MASK = [x for i in range(16) for x in (2 * i + 1, 2 * i)]


def _head_cut(slope8):
    c = 26.0 / slope8
    return min(N, 128 * math.ceil((c + 1) / 128) + 128)


def kept_j_chunks(cut, i0):
    return [j for j in range(NJ) if i0 - (j * 128 + 127) <= cut]


def build_program(dbg=False):
    nc = bacc.Bacc("TRN2", target_bir_lowering=False, debug=False,
                   num_devices=N_CORES)
    dbg_outs = {}

    def dbg_tensor(name, shape, dt_=F32):
        dbg_outs[name] = nc.dram_tensor(name, shape, dt_, kind="ExternalOutput")
        return dbg_outs[name]

    xT = nc.dram_tensor("xT", [C, N], BF, kind="ExternalInput")
    wT = nc.dram_tensor("wT", [C, 768], BF, kind="ExternalInput")
    ctq = nc.dram_tensor("ctq", [128, N], F32, kind="ExternalInput")
    stq = nc.dram_tensor("stq", [128, N], F32, kind="ExternalInput")
    ctk = nc.dram_tensor("ctk", [128, N], F32, kind="ExternalInput")
    stk = nc.dram_tensor("stk", [128, N], F32, kind="ExternalInput")
    ebias = nc.dram_tensor("ebias", [G, 128, EB_W], BF, kind="ExternalInput")
    pwT = nc.dram_tensor("pwT", [2 * C, C], BF, kind="ExternalInput")
    pb = nc.dram_tensor("pb", [128, 8], F32, kind="ExternalInput")
    out = nc.dram_tensor("out", [C, 512], F32, kind="ExternalOutput")

    with tile.TileContext(nc) as tc:
        with tc.tile_pool(name="persist", bufs=1) as pp, \
             tc.tile_pool(name="work", bufs=1) as wp, \
             tc.tile_pool(name="psum", bufs=1, space="PSUM") as psp, \
             tc.tile_pool(name="dram", bufs=1, space="DRAM") as dp:

            # ---- persistent tiles (live across phases) ----
            qkp = [pp.tile([128, N], BF, name=f"qk{m}") for m in range(4)]
            vts = [pp.tile([128, G * 65], BF, name=f"vt{j}") for j in range(NJ)]
            yts = [pp.tile([128, N], BF, name=f"yt{i}") for i in range(2)]
            pbt = pp.tile([128, 8], F32, name="pbt")
            nc.sync.dma_start(pbt[:], pb.ap()[:, :])

            with tc.tile_pool(name="ph1", bufs=1) as p1:
                # ---- phase-1 inputs ----
                xts = []
                for ci in range(8):
                    t = p1.tile([128, N], BF, name=f"xt{ci}")
                    nc.sync.dma_start(t[:], xT.ap()[ci * 128:(ci + 1) * 128, :])
                    xts.append(t)
                wts = []
                for ci in range(8):
                    t = p1.tile([128, 768], BF, name=f"wt{ci}")
                    nc.sync.dma_start(t[:], wT.ap()[ci * 128:(ci + 1) * 128, :])
                    wts.append(t)
                tabs = {}
                for nm, dt_ in (("ctq", ctq), ("stq", stq), ("ctk", ctk),
                                ("stk", stk)):
                    t = p1.tile([128, N], F32, name=f"tab_{nm}")
                    nc.sync.dma_start(t[:], dt_.ap()[:, :])
                    tabs[nm] = t

                # ---- QKV (q/k) + RoPE ----
                # wT cols: q01 | q23 | k01 | k23 | v (4x64)
                sc_qkv = nc.enter_named_scope("qkv", False)
                for m in range(4):
                    ct = tabs["ctq"] if m < 2 else tabs["ctk"]
                    st = tabs["stq"] if m < 2 else tabs["stk"]
                    for isl in range(NI):
                        sl = slice(isl * 512, (isl + 1) * 512)
                        ps = psp.tile([128, 512], F32, name="qkvps",
                                      tag="s", bufs=4)
                        for ci in range(8):
                            nc.tensor.matmul(ps[:],
                                             wts[ci][:, m * 128:(m + 1) * 128],
                                             xts[ci][:, sl],
                                             start=(ci == 0), stop=(ci == 7))
                        rot = wp.tile([128, 512], F32, name="rot", tag="rot",
                                      bufs=2)
                        nc.vector.stream_shuffle(rot[:], ps[:], SHUF_MASK)
                        t1 = wp.tile([128, 512], F32, name="ropet1",
                                     tag="ropet1", bufs=2)
                        nc.vector.tensor_mul(t1[:], rot[:], st[:, sl])
                        t2 = wp.tile([128, 512], F32, name="ropet2",
                                     tag="ropet2", bufs=2)
                        nc.vector.tensor_mul(t2[:], ps[:], ct[:, sl])
                        nc.vector.tensor_add(qkp[m][:, sl], t2[:], t1[:])

                nc.leave_named_scope("qkv", sc_qkv[0], False)
                # ---- V (non-transposed, direct) ----
                sc_v = nc.enter_named_scope("vdir", False)
                for j in range(NJ):
                    pv = psp.tile([128, 256], F32, name="vps",
                                  tag=f"oT{j % 2}", bufs=1)
                    for ci in range(8):
                        nc.tensor.matmul(pv[:], xts[ci][:, j * 128:(j + 1) * 128],
                                         wts[ci][:, 512:768],
                                         start=(ci == 0), stop=(ci == 7))
                    vt = vts[j]
                    vt_v = vt[:].rearrange("p (h e) -> p h e", e=65)
                    nc.vector.tensor_copy(vt_v[:, :, 0:64],
                                          pv[:].rearrange("p (h e) -> p h e",
                                                          e=64))
                    nc.vector.memset(vt_v[:, :, 64:65], 1.0)
                nc.leave_named_scope("vdir", sc_v[0], False)

            # ---- attention per head slot (j-outer: stationary k/v reuse) ----
            ebs = []
            ebpool = tc.tile_pool(name="ph2", bufs=1)
            p2 = ebpool.__enter__()
            for s in range(G):
                t = p2.tile([128, EB_W], BF, name=f"eb{s}")
                nc.sync.dma_start(t[:], ebias.ap()[s, :, :])
                ebs.append(t)
            for slot in range(G):
                qh = qkp[slot // 2][(slot % 2) * 64:(slot % 2) * 64 + 64, :]
                kh = qkp[2 + slot // 2][(slot % 2) * 64:(slot % 2) * 64 + 64, :]
                cut = SLOT_CUT[slot]
                ebt = ebs[slot]
                with nc.named_scope(f"attn{slot}"):
                    kept_per_isl = [kept_j_chunks(cut, isl * 512)
                                    for isl in range(NI)]
                    oTs = [psp.tile([65, 512], F32, name=f"oT{isl}",
                                    tag=f"oT{isl}", bufs=1,
                                    padded_shape=[128, 512])
                           for isl in range(NI)]
                    for j in range(NJ):
                        isls = [isl for isl in range(NI) if j in kept_per_isl[isl]]
                        ps_list = []
                        for isl in isls:
                            i0 = isl * 512
                            j0 = j * 128
                            s = psp.tile([128, 512], F32, name="s", tag="s",
                                         bufs=4)
                            nc.tensor.matmul(s[:], kh[:, j0:j0 + 128],
                                             qh[:, i0:i0 + 512],
                                             start=True, stop=True)
                            p = wp.tile([128, 512], BF, name="p", tag="p",
                                        bufs=6)
                            nc.scalar.activation(
                                p[:], s[:], mybir.ActivationFunctionType.Exp)
                            if j0 - i0 < 512:
                                off = i0 - j0 + EB_W // 2 - 1
                                nc.vector.tensor_mul(p[:], p[:],
                                                     ebt[:, off:off + 512])
                            ps_list.append((isl, p))
                        for isl, p in ps_list:
                            j_kept = kept_per_isl[isl]
                            nc.tensor.matmul(
                                oTs[isl][:], vts[j][:, slot * 65:slot * 65 + 65],
                                p[:], start=(j == j_kept[0]),
                                stop=(j == j_kept[-1]))
                    for isl in range(NI):
                        i0 = isl * 512
                        oT = oTs[isl]
                        den = wp.tile([1, 512], F32, name="den", tag="den",
                                      bufs=2)
                        nc.vector.tensor_copy(den[:], oT[64:65, :])
                        rec = wp.tile([1, 512], F32, name="rec", tag="rec",
                                      bufs=2)
                        nc.vector.reciprocal_approx_fast(rec[:], den[:])
                        R = wp.tile([64, 512], F32, name="R", tag="R", bufs=2)
                        nc.gpsimd.partition_broadcast(R[:], rec[:])
                        yt = yts[slot // 2]
                        r0 = (slot % 2) * 64
                        nc.vector.tensor_mul(yt[r0:r0 + 64, i0:i0 + 512],
                                             oT[0:64, :], R[:])
            ebpool.__exit__(None, None, None)

            if dbg:
                for m in range(4):
                    t = dbg_tensor(f"dbg_qk{m}", [128, N], BF)
                    nc.sync.dma_start(t.ap()[:, :], qkp[m][:])
                t = dbg_tensor("dbg_vt0", [128, G * 65], BF)
                nc.sync.dma_start(t.ap()[:, :], vts[0][:])
                for i in range(2):
                    t = dbg_tensor(f"dbg_yt{i}", [128, N], BF)
                    nc.sync.dma_start(t.ap()[:, :], yts[i][:])

            sc_a2a = nc.enter_named_scope("a2a", False)
            # ---- AllToAll (8-core): shard j = (my heads, q-quarter j%4) ----
            # out shard i = core i's heads for my quarter; other-batch rows are
            # neutralized by zero rows in pwT.
            a2a_in = dp.tile([2048, 512], BF, name="a2a_in")
            a2a_out = dp.tile([2048, 512], BF, name="a2a_out")
            for shard in range(8):
                q = shard % 4
                for slot in range(G):
                    r = 256 * shard + 64 * slot
                    nc.sync.dma_start(
                        a2a_in[r:r + 64, :],
                        yts[slot // 2][(slot % 2) * 64:(slot % 2) * 64 + 64,
                                       q * 512:(q + 1) * 512])
            nc.gpsimd.collective_compute(
                "AllToAll", mybir.AluOpType.bypass, replica_groups=RG8,
                ins=[a2a_in.opt()], outs=[a2a_out.opt()])
            nc.leave_named_scope("a2a", sc_a2a[0], False)

            with tc.tile_pool(name="ph3", bufs=1) as p3:
                pwts = []
                for ci in range(16):
                    t = p3.tile([128, C], BF, name=f"pwt{ci}")
                    nc.sync.dma_start(t[:], pwT.ap()[ci * 128:(ci + 1) * 128, :])
                    pwts.append(t)
                ytf = []
                for ci in range(16):
                    t = p3.tile([128, 512], BF, name=f"ytf{ci}")
                    nc.sync.dma_start(t[:], a2a_out[ci * 128:(ci + 1) * 128, :])
                    ytf.append(t)

                # ---- projection: outT [co, my q-quarter] ----
                sc_pj = nc.enter_named_scope("proj", False)
                for co in range(8):
                    pj = psp.tile([128, 512], F32, name="pj", tag="s", bufs=4)
                    for ci in range(16):
                        nc.tensor.matmul(pj[:],
                                         pwts[ci][:, co * 128:(co + 1) * 128],
                                         ytf[ci][:], start=(ci == 0),
                                         stop=(ci == 15))
                    ot = wp.tile([128, 512], F32, name="ot", tag="ot", bufs=2)
                    nc.scalar.add(ot[:], pj[:], pbt[:, co:co + 1])
                    nc.sync.dma_start(out.ap()[co * 128:(co + 1) * 128, :], ot[:])
                nc.leave_named_scope("proj", sc_pj[0], False)

    nc.compile()
    return nc


def prep_inputs(x, qkv_w, proj_w, proj_b, slopes):
    """Build the 8 per-core input maps (all host-side numpy)."""
    x = np.asarray(x, np.float32)
    qkv_w = np.asarray(qkv_w, np.float32)
    proj_w = np.asarray(proj_w, np.float32)
    proj_b = np.asarray(proj_b, np.float32)
    slopes = np.asarray(slopes, np.float32)

    # RoPE tables (transposed [d, n], d pairwise-interleaved, x2 head copies)
    inv = 1.0 / (10000.0 ** (np.arange(0, D, 2, dtype=np.float32) / D))
    fr = np.arange(N, dtype=np.float32)[:, None] * inv[None, :]   # [N, 32]
    sin_t, cos_t = np.sin(fr), np.cos(fr)                          # [N, 32]
    ct64 = np.empty((64, N), np.float32)
    st64 = np.empty((64, N), np.float32)
    ct64[0::2] = cos_t.T
    ct64[1::2] = cos_t.T
    st64[0::2] = -sin_t.T      # row for first-half d: -sin
    st64[1::2] = sin_t.T       # row for second-half d: +sin
    ctq = np.vstack([ct64, ct64]) * SCALE
    stq = np.vstack([st64, st64]) * SCALE
    ctk = np.vstack([ct64, ct64])
    stk = np.vstack([st64, st64])

    pos_p = np.arange(128, dtype=np.float64)[:, None]
    t_off = np.arange(EB_W, dtype=np.float64)[None, :] - (EB_W // 2 - 1)
    dmin = np.minimum(pos_p - t_off, 0.0)  # j - i clipped

    in_maps = []
    for core in range(N_CORES):
        b = core // 4
        heads = GROUP_HEADS[core % 4]
        # wT: qkv_w rows for my heads, d-interleaved for q/k, transposed
        rows = []
        for kind in range(2):  # q, k (d-permuted)
            for h in heads:
                base = kind * C + h * D
                rows.extend(base + p for p in D_PERM)
        for h in heads:        # v (natural d order)
            rows.extend(2 * C + h * D + d for d in range(D))
        w_sel = qkv_w[rows, :]                     # [768, 1024]
        wT_c = np.ascontiguousarray(w_sel.T)       # [1024, 768]

        eb_c = np.empty((G, 128, EB_W), np.float32)
        for s, h in enumerate(heads):
            eb_c[s] = np.exp(float(slopes[h]) * MAX_BIAS * dmin)

        # pwT rows (ci) ordered as A2A output: shard i = core i (batch i//4,
        # group i%4), heads in slot order, d natural. Other-batch rows zero.
        pwT_c = np.zeros((2 * C, C), np.float32)
        for src in range(N_CORES):
            if src // 4 != b:
                continue
            for slot, h in enumerate(GROUP_HEADS[src % 4]):
                r = 256 * src + 64 * slot
                pwT_c[r:r + 64, :] = proj_w[:, h * D:(h + 1) * D].T
        pb_c = np.ascontiguousarray(proj_b.reshape(8, 128).T)  # [128, 8]

        in_maps.append({
            "xT": np.ascontiguousarray(x[b].T).astype(NPBF),
            "wT": wT_c.astype(NPBF),
            "ctq": ctq, "stq": stq, "ctk": ctk, "stk": stk,
            "ebias": eb_c.astype(NPBF),
            "pwT": pwT_c.astype(NPBF),
            "pb": pb_c,
        })
    return in_maps


_NC = None


def _get_nc():
    global _NC
    if _NC is None:
        _NC = build_program()
    return _NC


def run(inputs, trace=False):
    nc = _get_nc()
    in_maps = prep_inputs(**inputs)
    res = run_bass_kernel_spmd(nc, in_maps, core_ids=list(range(N_CORES)),
                               trace=trace)
    out = np.empty((B, N, C), np.float32)
    for core in range(N_CORES):
        b, g = core // 4, core % 4
        out[b, g * 512:(g + 1) * 512, :] = res.results[core]["out"].T
    return out, res


def kernel(**inputs) -> np.ndarray:
    out, _ = run(inputs, trace=False)
    return out


# revision 30
# speedup vs baseline: 1.7524x; 1.0445x over previous
"""Trainium2 Bass kernel: attention block (QKV + RoPE + ALiBi attention + proj).

Problem shapes: x [2, 2048, 1024], 16 heads x 64 dim, f32 I/O.
Sharding: batch (2) x head-groups (4 heads/core) = 8 cores; AllToAll regroups
per-head outputs into per-query-quarter shards for the output projection.

Dataflow is fully transposed on-chip:
  xT [C, N] --QKV--> qT/kT [D, N] (RoPE'd, d pairwise-interleaved) and v [N, D]
  sT = kT_chunk.T @ qT   [j, i]  -> p = exp(sT) * ebias_table  (no max-sub)
  oT = v_aug.T @ p       [d+1, i] (ones column gives softmax denominators)
  yT = oT[:64] * (1/denom broadcast)  -> A2A -> outT = pwT.T @ yT + b
Host transposes final outT back. All matmuls bf16 with f32 PSUM accumulation.
"""
import sys
if "/opt/trn_rl_repo" not in sys.path:
    sys.path.insert(0, "/opt/trn_rl_repo")

import math
import numpy as np
import ml_dtypes

import concourse.bass as bass
import concourse.mybir as mybir
import concourse.tile as tile
from concourse import bacc
from concourse.bass_utils import run_bass_kernel_spmd

BF = mybir.dt.bfloat16
F32 = mybir.dt.float32
NPBF = ml_dtypes.bfloat16

B, N, C = 2, 2048, 1024
H, D = 16, 64
G = 4                       # heads per core
N_CORES = 8
MAX_BIAS = 8.0
SCALE = D ** -0.5
RG8 = [[0, 1, 2, 3, 4, 5, 6, 7]]

# head dealing (snake by per-head band cost) and per-slot band cutoffs.
# Slot cut must be >= the cut of every head assigned to that slot.
GROUP_HEADS = [[15, 8, 7, 0], [14, 9, 6, 1], [13, 10, 5, 2], [12, 11, 4, 3]]
SLOT_CUT = [1024, 384, 256, 256]

NJ = N // 128               # 16 j-chunks
NI = N // 512               # 4 i-slices
EB_W = 4096                 # ebias table width (offset t = i - j + 2047)

# d-permutation inside each head: [0, 32, 1, 33, ...] so rotate_half becomes
# an even/odd partition swap (StreamShuffle-able within 32-row quadrants).
D_PERM = [x for i in range(32) for x in (i, i + 32)]
SHUF_MASK = [x for i in range(16) for x in (2 * i + 1, 2 * i)]


def _head_cut(slope8):
    c = 26.0 / slope8
    return min(N, 128 * math.ceil((c + 1) / 128) + 128)


def kept_j_chunks(cut, i0):
    return [j for j in range(NJ) if i0 - (j * 128 + 127) <= cut]


def build_program(dbg=False):
    nc = bacc.Bacc("TRN2", target_bir_lowering=False, debug=False,
                   num_devices=N_CORES)
    dbg_outs = {}

    def dbg_tensor(name, shape, dt_=F32):
        dbg_outs[name] = nc.dram_tensor(name, shape, dt_, kind="ExternalOutput")
        return dbg_outs[name]

    xT = nc.dram_tensor("xT", [C, N], BF, kind="ExternalInput")
    wT = nc.dram_tensor("wT", [C, 768], BF, kind="ExternalInput")
    ctq = nc.dram_tensor("ctq", [128, N], F32, kind="ExternalInput")
    stq = nc.dram_tensor("stq", [128, N], F32, kind="ExternalInput")
    ctk = nc.dram_tensor("ctk", [128, N], F32, kind="ExternalInput")
    stk = nc.dram_tensor("stk", [128, N], F32, kind="ExternalInput")
    ebias = nc.dram_tensor("ebias", [G, 128, EB_W], BF, kind="ExternalInput")
    pwT = nc.dram_tensor("pwT", [2 * C, C], BF, kind="ExternalInput")
    pb = nc.dram_tensor("pb", [128, 8], F32, kind="ExternalInput")
    out = nc.dram_tensor("out", [C, 512], F32, kind="ExternalOutput")

    with tile.TileContext(nc) as tc:
        with tc.tile_pool(name="persist", bufs=1) as pp, \
             tc.tile_pool(name="work", bufs=1) as wp, \
             tc.tile_pool(name="psum", bufs=1, space="PSUM") as psp, \
             tc.tile_pool(name="dram", bufs=1, space="DRAM") as dp:

            # ---- persistent tiles (live across phases) ----
            qkp = [pp.tile([128, N], BF, name=f"qk{m}") for m in range(4)]
            vts = [pp.tile([128, 2 * G * 65], BF, name=f"vt{j}")
                   for j in range(NJ // 2)]

            def v_ap(j, slot):
                # stationary v_aug [128, 65] for (j-chunk, slot)
                return vts[j // 2][:, (j % 2) * G * 65 + slot * 65:
                                   (j % 2) * G * 65 + slot * 65 + 65]
            yts = [pp.tile([128, N], BF, name=f"yt{i}") for i in range(2)]
            pbt = pp.tile([128, 8], F32, name="pbt")
            nc.sync.dma_start(pbt[:], pb.ap()[:, :])

            with tc.tile_pool(name="ph1", bufs=1) as p1:
                # ---- phase-1 inputs ----
                xts = []
                for ci in range(8):
                    t = p1.tile([128, N], BF, name=f"xt{ci}")
                    nc.sync.dma_start(t[:], xT.ap()[ci * 128:(ci + 1) * 128, :])
                    xts.append(t)
                wts = []
                for ci in range(8):
                    t = p1.tile([128, 768], BF, name=f"wt{ci}")
                    nc.sync.dma_start(t[:], wT.ap()[ci * 128:(ci + 1) * 128, :])
                    wts.append(t)
                tabs = {}
                for nm, dt_ in (("ctq", ctq), ("stq", stq), ("ctk", ctk),
                                ("stk", stk)):
                    t = p1.tile([128, N], F32, name=f"tab_{nm}")
                    nc.sync.dma_start(t[:], dt_.ap()[:, :])
                    tabs[nm] = t

                # ---- QKV (q/k) + RoPE ----
                # wT cols: q01 | q23 | k01 | k23 | v (4x64)
                sc_qkv = nc.enter_named_scope("qkv", False)
                for m in range(4):
                    ct = tabs["ctq"] if m < 2 else tabs["ctk"]
                    st = tabs["stq"] if m < 2 else tabs["stk"]
                    for ip in range(NI // 2):       # isl pairs, wide tiles
                        sl = slice(ip * 1024, (ip + 1) * 1024)
                        ps = psp.tile([128, 1024], F32, name="qkvps",
                                      tag="s", bufs=2)
                        for ci in range(8):
                            for h in range(2):
                                hs = slice((2 * ip + h) * 512,
                                           (2 * ip + h + 1) * 512)
                                nc.tensor.matmul(
                                    ps[:, h * 512:(h + 1) * 512],
                                    wts[ci][:, m * 128:(m + 1) * 128],
                                    xts[ci][:, hs],
                                    start=(ci == 0), stop=(ci == 7))
                        rot = wp.tile([128, 1024], F32, name="rot", tag="rot",
                                      bufs=2)
                        nc.vector.stream_shuffle(rot[:], ps[:], SHUF_MASK)
                        t1 = wp.tile([128, 1024], F32, name="ropet1",
                                     tag="ropet1", bufs=2)
                        nc.vector.tensor_mul(t1[:], rot[:], st[:, sl])
                        t2 = wp.tile([128, 1024], F32, name="ropet2",
                                     tag="ropet2", bufs=2)
                        nc.vector.tensor_mul(t2[:], ps[:], ct[:, sl])
                        nc.vector.tensor_add(qkp[m][:, sl], t2[:], t1[:])

                nc.leave_named_scope("qkv", sc_qkv[0], False)
                # ---- V (non-transposed, direct; 2 j-chunks per psum) ----
                sc_v = nc.enter_named_scope("vdir", False)
                for jp in range(NJ // 2):
                    pv = psp.tile([128, 512], F32, name="vps",
                                  tag=f"oT{jp % 2}", bufs=1)
                    for h in range(2):
                        j = 2 * jp + h
                        for ci in range(8):
                            nc.tensor.matmul(
                                pv[:, h * 256:(h + 1) * 256],
                                xts[ci][:, j * 128:(j + 1) * 128],
                                wts[ci][:, 512:768],
                                start=(ci == 0), stop=(ci == 7))
                    vt = vts[jp]
                    vt_v = vt[:].rearrange("p (a h e) -> p a h e", a=2, e=65)
                    nc.vector.tensor_copy(
                        vt_v[:, :, :, 0:64],
                        pv[:].rearrange("p (a h e) -> p a h e", a=2, e=64))
                    nc.vector.memset(vt_v[:, :, :, 64:65], 1.0)
                nc.leave_named_scope("vdir", sc_v[0], False)

            # ---- attention per head slot (j-outer: stationary k/v reuse) ----
            ebs = []
            ebpool = tc.tile_pool(name="ph2", bufs=1)
            p2 = ebpool.__enter__()
            for s in range(G):
                t = p2.tile([128, EB_W], BF, name=f"eb{s}")
                nc.sync.dma_start(t[:], ebias.ap()[s, :, :])
                ebs.append(t)
            a2a_ins, a2a_outs = [], []
            for slot in range(G):
                qh = qkp[slot // 2][(slot % 2) * 64:(slot % 2) * 64 + 64, :]
                kh = qkp[2 + slot // 2][(slot % 2) * 64:(slot % 2) * 64 + 64, :]
                cut = SLOT_CUT[slot]
                ebt = ebs[slot]
                with nc.named_scope(f"attn{slot}"):
                    kept_per_isl = [kept_j_chunks(cut, isl * 512)
                                    for isl in range(NI)]
                    oTs = [psp.tile([65, 512], F32, name=f"oT{isl}",
                                    tag=f"oT{isl}", bufs=1,
                                    padded_shape=[128, 512])
                           for isl in range(NI)]
                    for j in range(NJ):
                        j0 = j * 128
                        isls = [isl for isl in range(NI) if j in kept_per_isl[isl]]
                        # pair consecutive i-slices into wide tiles
                        groups = [isls[k:k + 2] for k in range(0, len(isls), 2)]
                        ps_list = []
                        for grp in groups:
                            w = 512 * len(grp)
                            i0 = grp[0] * 512
                            s = psp.tile([128, 1024], F32, name="s", tag="s",
                                         bufs=2)
                            for h, isl in enumerate(grp):
                                nc.tensor.matmul(
                                    s[:, h * 512:(h + 1) * 512],
                                    kh[:, j0:j0 + 128],
                                    qh[:, isl * 512:(isl + 1) * 512],
                                    start=True, stop=True)
                            p = wp.tile([128, 1024], BF, name="p", tag="p",
                                        bufs=4)
                            nc.scalar.activation(
                                p[:, 0:w], s[:, 0:w],
                                mybir.ActivationFunctionType.Exp)
                            if j0 - (grp[-1] * 512) < 512:
                                off = i0 - j0 + EB_W // 2 - 1
                                nc.vector.tensor_mul(p[:, 0:w], p[:, 0:w],
                                                     ebt[:, off:off + w])
                            ps_list.append((grp, p))
                        for grp, p in ps_list:
                            for h, isl in enumerate(grp):
                                j_kept = kept_per_isl[isl]
                                nc.tensor.matmul(
                                    oTs[isl][:], v_ap(j, slot),
                                    p[:, h * 512:(h + 1) * 512],
                                    start=(j == j_kept[0]),
                                    stop=(j == j_kept[-1]))
                    for isl in range(NI):
                        i0 = isl * 512
                        oT = oTs[isl]
                        den = wp.tile([1, 512], F32, name="den", tag="den",
                                      bufs=2)
                        nc.vector.tensor_copy(den[:], oT[64:65, :])
                        rec = wp.tile([1, 512], F32, name="rec", tag="rec",
                                      bufs=2)
                        nc.vector.reciprocal_approx_fast(rec[:], den[:])
                        R = wp.tile([64, 512], F32, name="R", tag="R", bufs=2)
                        nc.gpsimd.partition_broadcast(R[:], rec[:])
                        yt = yts[slot // 2]
                        r0 = (slot % 2) * 64
                        nc.vector.tensor_mul(yt[r0:r0 + 64, i0:i0 + 512],
                                             oT[0:64, :], R[:])
                if slot % 2 == 1:
                    # fire the half-A2A for the completed slot pair (0,1)/(2,3)
                    half = slot // 2
                    a2a_in = dp.tile([1024, 512], BF, name=f"a2a_in{half}")
                    a2a_ins.append(a2a_in)
                    a2a_out = dp.tile([1024, 512], BF, name=f"a2a_out{half}")
                    a2a_outs.append(a2a_out)
                    for shard in range(8):
                        q = shard % 4
                        for sl2 in range(2):
                            r = 128 * shard + 64 * sl2
                            nc.sync.dma_start(
                                a2a_in[r:r + 64, :],
                                yts[half][sl2 * 64:sl2 * 64 + 64,
                                          q * 512:(q + 1) * 512])
                    with nc.named_scope(f"a2a{half}"):
                        nc.gpsimd.collective_compute(
                            "AllToAll", mybir.AluOpType.bypass,
                            replica_groups=RG8,
                            ins=[a2a_in.opt()], outs=[a2a_out.opt()])
            ebpool.__exit__(None, None, None)

            if dbg:
                for m in range(4):
                    t = dbg_tensor(f"dbg_qk{m}", [128, N], BF)
                    nc.sync.dma_start(t.ap()[:, :], qkp[m][:])
                t = dbg_tensor("dbg_vt0", [128, 2 * G * 65], BF)
                nc.sync.dma_start(t.ap()[:, :], vts[0][:])
                for i in range(2):
                    t = dbg_tensor(f"dbg_yt{i}", [128, N], BF)
                    nc.sync.dma_start(t.ap()[:, :], yts[i][:])

            with tc.tile_pool(name="ph3", bufs=1) as p3:
                pwts = []
                for ci in range(16):
                    t = p3.tile([128, C], BF, name=f"pwt{ci}")
                    nc.sync.dma_start(t[:], pwT.ap()[ci * 128:(ci + 1) * 128, :])
                    pwts.append(t)
                ytf = []
                for half in range(2):
                    for ci in range(8):
                        t = p3.tile([128, 512], BF, name=f"ytf{half}_{ci}")
                        nc.sync.dma_start(
                            t[:], a2a_outs[half][ci * 128:(ci + 1) * 128, :])
                        ytf.append(t)

                # ---- projection: outT [co, my q-quarter] ----
                sc_pj = nc.enter_named_scope("proj", False)
                for co in range(8):
                    pj = psp.tile([128, 512], F32, name="pj", tag="s", bufs=2)
                    for ci in range(16):
                        nc.tensor.matmul(pj[:],
                                         pwts[ci][:, co * 128:(co + 1) * 128],
                                         ytf[ci][:], start=(ci == 0),
                                         stop=(ci == 15))
                    ot = wp.tile([128, 512], F32, name="ot", tag="ot", bufs=2)
                    nc.scalar.add(ot[:], pj[:], pbt[:, co:co + 1])
                    nc.sync.dma_start(out.ap()[co * 128:(co + 1) * 128, :], ot[:])
                nc.leave_named_scope("proj", sc_pj[0], False)

    nc.compile()
    return nc


def prep_inputs(x, qkv_w, proj_w, proj_b, slopes):
    """Build the 8 per-core input maps (all host-side numpy)."""
    x = np.asarray(x, np.float32)
    qkv_w = np.asarray(qkv_w, np.float32)
    proj_w = np.asarray(proj_w, np.float32)
    proj_b = np.asarray(proj_b, np.float32)
    slopes = np.asarray(slopes, np.float32)

    # RoPE tables (transposed [d, n], d pairwise-interleaved, x2 head copies)
    inv = 1.0 / (10000.0 ** (np.arange(0, D, 2, dtype=np.float32) / D))
    fr = np.arange(N, dtype=np.float32)[:, None] * inv[None, :]   # [N, 32]
    sin_t, cos_t = np.sin(fr), np.cos(fr)                          # [N, 32]
    ct64 = np.empty((64, N), np.float32)
    st64 = np.empty((64, N), np.float32)
    ct64[0::2] = cos_t.T
    ct64[1::2] = cos_t.T
    st64[0::2] = -sin_t.T      # row for first-half d: -sin
    st64[1::2] = sin_t.T       # row for second-half d: +sin
    ctq = np.vstack([ct64, ct64]) * SCALE
    stq = np.vstack([st64, st64]) * SCALE
    ctk = np.vstack([ct64, ct64])
    stk = np.vstack([st64, st64])

    pos_p = np.arange(128, dtype=np.float64)[:, None]
    t_off = np.arange(EB_W, dtype=np.float64)[None, :] - (EB_W // 2 - 1)
    dmin = np.minimum(pos_p - t_off, 0.0)  # j - i clipped

    in_maps = []
    for core in range(N_CORES):
        b = core // 4
        heads = GROUP_HEADS[core % 4]
        # wT: qkv_w rows for my heads, d-interleaved for q/k, transposed
        rows = []
        for kind in range(2):  # q, k (d-permuted)
            for h in heads:
                base = kind * C + h * D
                rows.extend(base + p for p in D_PERM)
        for h in heads:        # v (natural d order)
            rows.extend(2 * C + h * D + d for d in range(D))
        w_sel = qkv_w[rows, :]                     # [768, 1024]
        wT_c = np.ascontiguousarray(w_sel.T)       # [1024, 768]

        eb_c = np.empty((G, 128, EB_W), np.float32)
        for s, h in enumerate(heads):
            eb_c[s] = np.exp(float(slopes[h]) * MAX_BIAS * dmin)

        # pwT rows (ci) ordered as the two half-A2A outputs: half h rows =
        # [core0 (slot 2h, 2h+1), core1 ..., core7]. Other-batch rows zero.
        pwT_c = np.zeros((2 * C, C), np.float32)
        for half in range(2):
            for src in range(N_CORES):
                if src // 4 != b:
                    continue
                for sl2 in range(2):
                    h = GROUP_HEADS[src % 4][2 * half + sl2]
                    r = 1024 * half + 128 * src + 64 * sl2
                    pwT_c[r:r + 64, :] = proj_w[:, h * D:(h + 1) * D].T
        pb_c = np.ascontiguousarray(proj_b.reshape(8, 128).T)  # [128, 8]

        in_maps.append({
            "xT": np.ascontiguousarray(x[b].T).astype(NPBF),
            "wT": wT_c.astype(NPBF),
            "ctq": ctq, "stq": stq, "ctk": ctk, "stk": stk,
            "ebias": eb_c.astype(NPBF),
            "pwT": pwT_c.astype(NPBF),
            "pb": pb_c,
        })
    return in_maps


_NC = None


def _get_nc():
    global _NC
    if _NC is None:
        _NC = build_program()
    return _NC


def run(inputs, trace=False):
    nc = _get_nc()
    in_maps = prep_inputs(**inputs)
    res = run_bass_kernel_spmd(nc, in_maps, core_ids=list(range(N_CORES)),
                               trace=trace)
    out = np.empty((B, N, C), np.float32)
    for core in range(N_CORES):
        b, g = core // 4, core % 4
        out[b, g * 512:(g + 1) * 512, :] = res.results[core]["out"].T
    return out, res


def kernel(**inputs) -> np.ndarray:
    out, _ = run(inputs, trace=False)
    return out


# revision 33
# speedup vs baseline: 2.0104x; 1.1472x over previous
"""Trainium2 Bass kernel: attention block (QKV + RoPE + ALiBi attention + proj).

Problem shapes: x [2, 2048, 1024], 16 heads x 64 dim, f32 I/O.
Sharding: batch (2) x head-groups (4 heads/core) = 8 cores. Two 4-core
AllGathers exchange per-head outputs; each core then gathers its own
query-quarter columns via indirect DMA (per-core indices are input data,
keeping the SPMD program uniform) and runs the output projection.

Dataflow is fully transposed on-chip:
  xT [C, N] --QKV--> qT/kT [D, N] (RoPE'd, d pairwise-interleaved) and v [N, D]
  sT = kT_chunk.T @ qT   [j, i]  -> p = exp(sT) * ebias_table  (no max-sub)
  oT = v_aug.T @ p       [d+1, i] (ones column gives softmax denominators)
  yT = oT[:64] * (1/denom broadcast) -> AG -> outT = pwT.T @ yT[:, my q] + b
Host transposes final outT back. All matmuls bf16 with f32 PSUM accumulation.
ALiBi is applied multiplicatively (exp(slope*min(j-i,0)) Toeplitz tables) and
far-past tiles below ~1e-3 relative mass are skipped per-slot (SLOT_CUT).
"""
import sys
if "/opt/trn_rl_repo" not in sys.path:
    sys.path.insert(0, "/opt/trn_rl_repo")

import numpy as np
import ml_dtypes

import concourse.bass as bass
import concourse.mybir as mybir
import concourse.tile as tile
from concourse import bacc
from concourse.bass_utils import run_bass_kernel_spmd

BF = mybir.dt.bfloat16
F32 = mybir.dt.float32
NPBF = ml_dtypes.bfloat16

B, N, C = 2, 2048, 1024
H, D = 16, 64
G = 4                       # heads per core
N_CORES = 8
MAX_BIAS = 8.0
SCALE = D ** -0.5
RG4 = [[0, 1, 2, 3], [4, 5, 6, 7]]

# head dealing (snake by per-head band cost) and per-slot band cutoffs
# (validated empirically: truncation error << bf16 error).
GROUP_HEADS = [[15, 8, 7, 0], [14, 9, 6, 1], [13, 10, 5, 2], [12, 11, 4, 3]]
SLOT_CUT = [384, 192, 128, 128]

NJ = N // 128               # 16 j-chunks
NI = N // 512               # 4 i-slices
EB_W = 2048                 # ebias table width; u = (i - j) + EB_OFF
EB_OFF = 1023

# d-permutation inside each head: [0, 32, 1, 33, ...] so rotate_half becomes
# an even/odd partition swap (StreamShuffle-able within 32-row quadrants).
D_PERM = [x for i in range(32) for x in (i, i + 32)]
SHUF_MASK = [x for i in range(16) for x in (2 * i + 1, 2 * i)]


def kept_j_chunks(cut, i0):
    return [j for j in range(NJ) if i0 - (j * 128 + 127) <= cut]


def build_program(dbg=False):
    nc = bacc.Bacc("TRN2", target_bir_lowering=False, debug=False,
                   num_devices=N_CORES)
    dbg_outs = {}

    def dbg_tensor(name, shape, dt_=F32):
        dbg_outs[name] = nc.dram_tensor(name, shape, dt_, kind="ExternalOutput")
        return dbg_outs[name]

    xT = nc.dram_tensor("xT", [C, N], BF, kind="ExternalInput")
    wT = nc.dram_tensor("wT", [C, 768], BF, kind="ExternalInput")
    ctq = nc.dram_tensor("ctq", [128, N], BF, kind="ExternalInput")
    stq = nc.dram_tensor("stq", [128, N], BF, kind="ExternalInput")
    ctk = nc.dram_tensor("ctk", [128, N], BF, kind="ExternalInput")
    stk = nc.dram_tensor("stk", [128, N], BF, kind="ExternalInput")
    ebias = nc.dram_tensor("ebias", [G, 128, EB_W], BF, kind="ExternalInput")
    pwT = nc.dram_tensor("pwT", [C, C], BF, kind="ExternalInput")
    pb = nc.dram_tensor("pb", [128, 8], F32, kind="ExternalInput")
    qidx = nc.dram_tensor("qidx", [128, 4], mybir.dt.uint32,
                          kind="ExternalInput")
    out = nc.dram_tensor("out", [C, 512], F32, kind="ExternalOutput")

    with tile.TileContext(nc) as tc:
        with tc.tile_pool(name="persist", bufs=1) as pp, \
             tc.tile_pool(name="work", bufs=1) as wp, \
             tc.tile_pool(name="psum", bufs=1, space="PSUM") as psp, \
             tc.tile_pool(name="dram", bufs=1, space="DRAM") as dp:

            # ---- persistent tiles (live across phases) ----
            qkp = [pp.tile([128, N], BF, name=f"qk{m}") for m in range(4)]
            vts = [pp.tile([128, 2 * G * 65], BF, name=f"vt{j}")
                   for j in range(NJ // 2)]

            def v_ap(j, slot):
                # stationary v_aug [128, 65] for (j-chunk, slot)
                base = (j % 2) * G * 65 + slot * 65
                return vts[j // 2][:, base:base + 65]

            yts = [pp.tile([128, N], BF, name=f"yt{i}") for i in range(2)]
            pbt = pp.tile([128, 8], F32, name="pbt")
            nc.sync.dma_start(pbt[:], pb.ap()[:, :])
            qit = pp.tile([128, 4], mybir.dt.uint32, name="qit")
            nc.sync.dma_start(qit[:], qidx.ap()[:, :])

            with tc.tile_pool(name="ph1", bufs=1) as p1:
                # ---- phase-1 inputs ----
                xts = []
                for ci in range(8):
                    t = p1.tile([128, N], BF, name=f"xt{ci}")
                    nc.sync.dma_start(t[:], xT.ap()[ci * 128:(ci + 1) * 128, :])
                    xts.append(t)
                wts = []
                for ci in range(8):
                    t = p1.tile([128, 768], BF, name=f"wt{ci}")
                    nc.sync.dma_start(t[:], wT.ap()[ci * 128:(ci + 1) * 128, :])
                    wts.append(t)
                tabs = {}
                for nm, dt_ in (("ctq", ctq), ("stq", stq), ("ctk", ctk),
                                ("stk", stk)):
                    t = p1.tile([128, N], BF, name=f"tab_{nm}")
                    nc.sync.dma_start(t[:], dt_.ap()[:, :])
                    tabs[nm] = t

                # ---- QKV (q/k) + RoPE ----
                # wT cols: q01 | q23 | k01 | k23 | v (4x64)
                sc = nc.enter_named_scope("qkv", False)
                for m in range(4):
                    ct = tabs["ctq"] if m < 2 else tabs["ctk"]
                    st = tabs["stq"] if m < 2 else tabs["stk"]
                    for ip in range(NI // 2):       # isl pairs, wide tiles
                        sl = slice(ip * 1024, (ip + 1) * 1024)
                        ps = psp.tile([128, 1024], F32, name="qkvps",
                                      tag="s", bufs=2)
                        for ci in range(8):
                            for hh in range(2):
                                hs = slice((2 * ip + hh) * 512,
                                           (2 * ip + hh + 1) * 512)
                                nc.tensor.matmul(
                                    ps[:, hh * 512:(hh + 1) * 512],
                                    wts[ci][:, m * 128:(m + 1) * 128],
                                    xts[ci][:, hs],
                                    start=(ci == 0), stop=(ci == 7))
                        cps = wp.tile([128, 1024], BF, name="cps", tag="cps",
                                      bufs=2)
                        nc.vector.tensor_copy(cps[:], ps[:])
                        rot = wp.tile([128, 1024], BF, name="rot", tag="rot",
                                      bufs=2)
                        nc.vector.stream_shuffle(rot[:], cps[:], SHUF_MASK)
                        t1 = wp.tile([128, 1024], BF, name="ropet1",
                                     tag="ropet1", bufs=2)
                        nc.vector.tensor_mul(t1[:], rot[:], st[:, sl])
                        t2 = wp.tile([128, 1024], BF, name="ropet2",
                                     tag="ropet2", bufs=2)
                        nc.vector.tensor_mul(t2[:], cps[:], ct[:, sl])
                        nc.vector.tensor_add(qkp[m][:, sl], t2[:], t1[:])
                nc.leave_named_scope("qkv", sc[0], False)

                # ---- V (non-transposed, direct; 2 j-chunks per psum) ----
                sc = nc.enter_named_scope("vdir", False)
                for jp in range(NJ // 2):
                    pv = psp.tile([128, 512], F32, name="vps",
                                  tag=f"oT{jp % 2}", bufs=1)
                    for hh in range(2):
                        j = 2 * jp + hh
                        for ci in range(8):
                            nc.tensor.matmul(
                                pv[:, hh * 256:(hh + 1) * 256],
                                xts[ci][:, j * 128:(j + 1) * 128],
                                wts[ci][:, 512:768],
                                start=(ci == 0), stop=(ci == 7))
                    vt_v = vts[jp][:].rearrange("p (a h e) -> p a h e",
                                                a=2, e=65)
                    nc.vector.tensor_copy(
                        vt_v[:, :, :, 0:64],
                        pv[:].rearrange("p (a h e) -> p a h e", a=2, e=64))
                    nc.vector.memset(vt_v[:, :, :, 64:65], 1.0)
                nc.leave_named_scope("vdir", sc[0], False)

            # ---- attention per head slot (j-outer: stationary k/v reuse) ----
            ebs = []
            ebpool = tc.tile_pool(name="ph2", bufs=1)
            p2 = ebpool.__enter__()
            for s in range(G):
                t = p2.tile([128, EB_W], BF, name=f"eb{s}")
                nc.sync.dma_start(t[:], ebias.ap()[s, :, :])
                ebs.append(t)
            ag_outs = []
            for slot in range(G):
                qh = qkp[slot // 2][(slot % 2) * 64:(slot % 2) * 64 + 64, :]
                kh = qkp[2 + slot // 2][(slot % 2) * 64:(slot % 2) * 64 + 64, :]
                cut = SLOT_CUT[slot]
                ebt = ebs[slot]
                with nc.named_scope(f"attn{slot}"):
                    kept_per_isl = [kept_j_chunks(cut, isl * 512)
                                    for isl in range(NI)]
                    oTs = [psp.tile([65, 512], F32, name=f"oT{isl}",
                                    tag=f"oT{isl}", bufs=1,
                                    padded_shape=[128, 512])
                           for isl in range(NI)]
                    for j in range(NJ):
                        j0 = j * 128
                        isls = [isl for isl in range(NI)
                                if j in kept_per_isl[isl]]
                        groups = [isls[k:k + 2] for k in range(0, len(isls), 2)]
                        ps_list = []
                        for grp in groups:
                            w = 512 * len(grp)
                            i0 = grp[0] * 512
                            s = psp.tile([128, 1024], F32, name="s", tag="s",
                                         bufs=2)
                            for hh, isl in enumerate(grp):
                                nc.tensor.matmul(
                                    s[:, hh * 512:(hh + 1) * 512],
                                    kh[:, j0:j0 + 128],
                                    qh[:, isl * 512:(isl + 1) * 512],
                                    start=True, stop=True)
                            p = wp.tile([128, 1024], BF, name="p", tag="p",
                                        bufs=4)
                            nc.scalar.activation(
                                p[:, 0:w], s[:, 0:w],
                                mybir.ActivationFunctionType.Exp)
                            if j0 - (grp[-1] * 512) < 512:
                                off = i0 - j0 + EB_OFF
                                nc.vector.tensor_mul(p[:, 0:w], p[:, 0:w],
                                                     ebt[:, off:off + w])
                            ps_list.append((grp, p))
                        for grp, p in ps_list:
                            for hh, isl in enumerate(grp):
                                j_kept = kept_per_isl[isl]
                                nc.tensor.matmul(
                                    oTs[isl][:], v_ap(j, slot),
                                    p[:, hh * 512:(hh + 1) * 512],
                                    start=(j == j_kept[0]),
                                    stop=(j == j_kept[-1]))
                    for isl in range(NI):
                        i0 = isl * 512
                        oT = oTs[isl]
                        den = wp.tile([1, 512], F32, name="den", tag="den",
                                      bufs=2)
                        nc.vector.tensor_copy(den[:], oT[64:65, :])
                        rec = wp.tile([1, 512], F32, name="rec", tag="rec",
                                      bufs=2)
                        nc.vector.reciprocal_approx_fast(rec[:], den[:])
                        R = wp.tile([64, 512], F32, name="R", tag="R", bufs=2)
                        nc.gpsimd.partition_broadcast(R[:], rec[:])
                        yt = yts[slot // 2]
                        r0 = (slot % 2) * 64
                        nc.vector.tensor_mul(yt[r0:r0 + 64, i0:i0 + 512],
                                             oT[0:64, :], R[:])
                if slot % 2 == 1:
                    # AllGather the completed slot pair (within batch group)
                    half = slot // 2
                    ag_in = dp.tile([128, N], BF, name=f"ag_in{half}")
                    ag_out = dp.tile([512, N], BF, name=f"ag_out{half}")
                    ag_outs.append(ag_out)
                    nc.sync.dma_start(ag_in[:, :], yts[half][:])
                    with nc.named_scope(f"ag{half}"):
                        nc.gpsimd.collective_compute(
                            "AllGather", mybir.AluOpType.bypass,
                            replica_groups=RG4,
                            ins=[ag_in.opt()], outs=[ag_out.opt()])
            ebpool.__exit__(None, None, None)

            if dbg:
                for m in range(4):
                    t = dbg_tensor(f"dbg_qk{m}", [128, N], BF)
                    nc.sync.dma_start(t.ap()[:, :], qkp[m][:])
                t = dbg_tensor("dbg_vt0", [128, 2 * G * 65], BF)
                nc.sync.dma_start(t.ap()[:, :], vts[0][:])
                for i in range(2):
                    t = dbg_tensor(f"dbg_yt{i}", [128, N], BF)
                    nc.sync.dma_start(t.ap()[:, :], yts[i][:])

            with tc.tile_pool(name="ph3", bufs=1) as p3:
                pwts = []
                for ci in range(8):
                    t = p3.tile([128, C], BF, name=f"pwt{ci}")
                    nc.sync.dma_start(t[:], pwT.ap()[ci * 128:(ci + 1) * 128, :])
                    pwts.append(t)
                # gather my query-quarter columns of the AG'd y (indices are
                # per-core input data -> SPMD-uniform program)
                ytf = []
                for half in range(2):
                    src = ag_outs[half][:].rearrange("r (q w) -> (r q) w",
                                                     w=512)
                    for c in range(4):
                        t = p3.tile([128, 512], BF, name=f"ytf{half}_{c}")
                        nc.gpsimd.indirect_dma_start(
                            out=t[:], out_offset=None, in_=src,
                            in_offset=bass.IndirectOffsetOnAxis(
                                ap=qit[:, c:c + 1], axis=0))
                        ytf.append(t)

                # ---- projection: outT [co, my q-quarter] ----
                sc = nc.enter_named_scope("proj", False)
                for co in range(8):
                    pj = psp.tile([128, 512], F32, name="pj", tag="s", bufs=2)
                    for ci in range(8):
                        nc.tensor.matmul(pj[:],
                                         pwts[ci][:, co * 128:(co + 1) * 128],
                                         ytf[ci][:], start=(ci == 0),
                                         stop=(ci == 7))
                    ot = wp.tile([128, 512], F32, name="ot", tag="ot", bufs=2)
                    nc.scalar.add(ot[:], pj[:], pbt[:, co:co + 1])
                    nc.sync.dma_start(out.ap()[co * 128:(co + 1) * 128, :],
                                      ot[:])
                nc.leave_named_scope("proj", sc[0], False)

    nc.compile()
    return nc


def prep_inputs(x, qkv_w, proj_w, proj_b, slopes):
    """Build the 8 per-core input maps (all host-side numpy)."""
    x = np.asarray(x, np.float32)
    qkv_w = np.asarray(qkv_w, np.float32)
    proj_w = np.asarray(proj_w, np.float32)
    proj_b = np.asarray(proj_b, np.float32)
    slopes = np.asarray(slopes, np.float32)

    # RoPE tables (transposed [d, n], d pairwise-interleaved, x2 head copies)
    inv = 1.0 / (10000.0 ** (np.arange(0, D, 2, dtype=np.float32) / D))
    fr = np.arange(N, dtype=np.float32)[:, None] * inv[None, :]   # [N, 32]
    sin_t, cos_t = np.sin(fr), np.cos(fr)
    ct64 = np.empty((64, N), np.float32)
    st64 = np.empty((64, N), np.float32)
    ct64[0::2] = cos_t.T
    ct64[1::2] = cos_t.T
    st64[0::2] = -sin_t.T
    st64[1::2] = sin_t.T
    ctq = (np.vstack([ct64, ct64]) * SCALE).astype(NPBF)
    stq = (np.vstack([st64, st64]) * SCALE).astype(NPBF)
    ctk = np.vstack([ct64, ct64]).astype(NPBF)
    stk = np.vstack([st64, st64]).astype(NPBF)

    pos_p = np.arange(128, dtype=np.float64)[:, None]
    t_off = np.arange(EB_W, dtype=np.float64)[None, :] - EB_OFF
    dmin = np.minimum(pos_p - t_off, 0.0)  # j - i clipped

    in_maps = []
    for core in range(N_CORES):
        b = core // 4
        g = core % 4
        heads = GROUP_HEADS[g]
        rows = []
        for kind in range(2):  # q, k (d-permuted)
            for h in heads:
                base = kind * C + h * D
                rows.extend(base + p for p in D_PERM)
        for h in heads:        # v (natural d order)
            rows.extend(2 * C + h * D + d for d in range(D))
        wT_c = np.ascontiguousarray(qkv_w[rows, :].T)      # [1024, 768]

        eb_c = np.empty((G, 128, EB_W), np.float32)
        for s, h in enumerate(heads):
            eb_c[s] = np.exp(float(slopes[h]) * MAX_BIAS * dmin)

        # pwT rows (ci) ordered as the two AG outputs: half h rows =
        # [rank0 (slot 2h, 2h+1), rank1, ..., rank3] x 64 d each.
        pwT_c = np.empty((C, C), np.float32)
        for half in range(2):
            for rank in range(4):
                for sl2 in range(2):
                    hh = GROUP_HEADS[rank][2 * half + sl2]
                    r = 512 * half + 128 * rank + 64 * sl2
                    pwT_c[r:r + 64, :] = proj_w[:, hh * D:(hh + 1) * D].T
        pb_c = np.ascontiguousarray(proj_b.reshape(8, 128).T)

        # quarter-gather row indices into ag_out viewed as [(r q) w]
        qidx_c = np.empty((128, 4), np.uint32)
        for c in range(4):
            qidx_c[:, c] = 512 * c + 4 * np.arange(128) + g

        in_maps.append({
            "xT": np.ascontiguousarray(x[b].T).astype(NPBF),
            "wT": wT_c.astype(NPBF),
            "ctq": ctq, "stq": stq, "ctk": ctk, "stk": stk,
            "ebias": eb_c.astype(NPBF),
            "pwT": pwT_c.astype(NPBF),
            "pb": pb_c,
            "qidx": qidx_c,
        })
    return in_maps


_NC = None


def _get_nc():
    global _NC
    if _NC is None:
        _NC = build_program()
    return _NC


def run(inputs, trace=False):
    nc = _get_nc()
    in_maps = prep_inputs(**inputs)
    res = run_bass_kernel_spmd(nc, in_maps, core_ids=list(range(N_CORES)),
                               trace=trace)
    out = np.empty((B, N, C), np.float32)
    for core in range(N_CORES):
        b, g = core // 4, core % 4
        out[b, g * 512:(g + 1) * 512, :] = res.results[core]["out"].T
    return out, res


def kernel(**inputs) -> np.ndarray:
    out, _ = run(inputs, trace=False)
    return out


# revision 36
# speedup vs baseline: 2.0971x; 1.0432x over previous
"""Trainium2 Bass kernel: attention block (QKV + RoPE + ALiBi attention + proj).

Problem shapes: x [2, 2048, 1024], 16 heads x 64 dim, f32 I/O.
Sharding: batch (2) x head-groups (4 heads/core) = 8 cores. Two 4-core
AllGathers exchange per-head outputs; each core then gathers its own
query-quarter columns via indirect DMA (per-core indices are input data,
keeping the SPMD program uniform) and runs the output projection.

Dataflow is fully transposed on-chip:
  xT [C, N] --QKV--> qT/kT [D, N] (RoPE'd, d pairwise-interleaved) and v [N, D]
  sT = kT_chunk.T @ qT   [j, i]  -> p = exp(sT) * ebias_table  (no max-sub)
  oT = v_aug.T @ p       [d+1, i] (ones column gives softmax denominators)
  yT = oT[:64] * (1/denom broadcast) -> AG -> outT = pwT.T @ yT[:, my q] + b
Host transposes final outT back. All matmuls bf16 with f32 PSUM accumulation.
ALiBi is applied multiplicatively (exp(slope*min(j-i,0)) Toeplitz tables) and
far-past tiles below ~1e-3 relative mass are skipped per-slot (SLOT_CUT).
"""
import sys
if "/opt/trn_rl_repo" not in sys.path:
    sys.path.insert(0, "/opt/trn_rl_repo")

import numpy as np
import ml_dtypes

import concourse.bass as bass
import concourse.mybir as mybir
import concourse.tile as tile
from concourse import bacc
from concourse.bass_utils import run_bass_kernel_spmd

BF = mybir.dt.bfloat16
F32 = mybir.dt.float32
NPBF = ml_dtypes.bfloat16

B, N, C = 2, 2048, 1024
H, D = 16, 64
G = 4                       # heads per core
N_CORES = 8
MAX_BIAS = 8.0
SCALE = D ** -0.5
RG4 = [[0, 1, 2, 3], [4, 5, 6, 7]]

# head dealing (snake by per-head band cost) and per-slot band cutoffs
# (validated empirically: truncation error << bf16 error).
GROUP_HEADS = [[15, 8, 7, 0], [14, 9, 6, 1], [13, 10, 5, 2], [12, 11, 4, 3]]
SLOT_CUT = [384, 192, 128, 128]

NJ = N // 128               # 16 j-chunks
NI = N // 512               # 4 i-slices
EB_W = 2048                 # ebias table width; u = (i - j) + EB_OFF
EB_OFF = 1023

# d-permutation inside each head: [0, 32, 1, 33, ...] so rotate_half becomes
# an even/odd partition swap (StreamShuffle-able within 32-row quadrants).
D_PERM = [x for i in range(32) for x in (i, i + 32)]
SHUF_MASK = [x for i in range(16) for x in (2 * i + 1, 2 * i)]


def kept_j_chunks(cut, i0):
    return [j for j in range(NJ) if i0 - (j * 128 + 127) <= cut]


def build_program(dbg=False):
    nc = bacc.Bacc("TRN2", target_bir_lowering=False, debug=False,
                   num_devices=N_CORES)
    dbg_outs = {}

    def dbg_tensor(name, shape, dt_=F32):
        dbg_outs[name] = nc.dram_tensor(name, shape, dt_, kind="ExternalOutput")
        return dbg_outs[name]

    xT = nc.dram_tensor("xT", [C, N], BF, kind="ExternalInput")
    wT = nc.dram_tensor("wT", [C, 768], BF, kind="ExternalInput")
    ctq = nc.dram_tensor("ctq", [128, N], BF, kind="ExternalInput")
    stq = nc.dram_tensor("stq", [128, N], BF, kind="ExternalInput")
    ctk = nc.dram_tensor("ctk", [128, N], BF, kind="ExternalInput")
    stk = nc.dram_tensor("stk", [128, N], BF, kind="ExternalInput")
    ebias = nc.dram_tensor("ebias", [G, 128, EB_W], BF, kind="ExternalInput")
    pwT = nc.dram_tensor("pwT", [C, C], BF, kind="ExternalInput")
    pb = nc.dram_tensor("pb", [128, 8], F32, kind="ExternalInput")
    qidx = nc.dram_tensor("qidx", [128, 4], mybir.dt.uint32,
                          kind="ExternalInput")
    out = nc.dram_tensor("out", [C, 512], F32, kind="ExternalOutput")

    with tile.TileContext(nc) as tc:
        with tc.tile_pool(name="persist", bufs=1) as pp, \
             tc.tile_pool(name="work", bufs=1) as wp, \
             tc.tile_pool(name="psum", bufs=1, space="PSUM") as psp, \
             tc.tile_pool(name="dram", bufs=1, space="DRAM") as dp:

            # ---- persistent tiles (live across phases) ----
            qkp = [pp.tile([128, N], BF, name=f"qk{m}") for m in range(4)]
            vts = [pp.tile([128, 2 * G * 65], BF, name=f"vt{j}")
                   for j in range(NJ // 2)]

            def v_ap(j, slot):
                # stationary v_aug [128, 65] for (j-chunk, slot)
                base = (j % 2) * G * 65 + slot * 65
                return vts[j // 2][:, base:base + 65]

            yts = [pp.tile([128, N], BF, name=f"yt{i}") for i in range(2)]
            pbt = pp.tile([128, 8], F32, name="pbt")
            nc.sync.dma_start(pbt[:], pb.ap()[:, :])
            qit = pp.tile([128, 4], mybir.dt.uint32, name="qit")
            nc.sync.dma_start(qit[:], qidx.ap()[:, :])
            # per-slot duplicated q (so QK's moving operand spans 128
            # partitions) and block-diag k stationaries (two 64-row j-chunks
            # of one head on the diagonal -> K=128 full-rate streaming)
            qds = [pp.tile([128, N], BF, name=f"qd{s}") for s in range(G)]
            bdb = [pp.tile([128, N], BF, name=f"bd{i}") for i in range(2)]
            nc.vector.memset(bdb[0][:], 0.0)
            nc.vector.memset(bdb[1][:], 0.0)

            with tc.tile_pool(name="ph1", bufs=1) as p1:
                # ---- phase-1 inputs ----
                xts = []
                for ci in range(8):
                    t = p1.tile([128, N], BF, name=f"xt{ci}")
                    nc.sync.dma_start(t[:], xT.ap()[ci * 128:(ci + 1) * 128, :])
                    xts.append(t)
                wts = []
                for ci in range(8):
                    t = p1.tile([128, 768], BF, name=f"wt{ci}")
                    nc.sync.dma_start(t[:], wT.ap()[ci * 128:(ci + 1) * 128, :])
                    wts.append(t)
                tabs = {}
                for nm, dt_ in (("ctq", ctq), ("stq", stq), ("ctk", ctk),
                                ("stk", stk)):
                    t = p1.tile([128, N], BF, name=f"tab_{nm}")
                    nc.sync.dma_start(t[:], dt_.ap()[:, :])
                    tabs[nm] = t

                # ---- QKV (q/k) + RoPE ----
                # wT cols: q01 | q23 | k01 | k23 | v (4x64)
                sc = nc.enter_named_scope("qkv", False)
                for m in range(4):
                    ct = tabs["ctq"] if m < 2 else tabs["ctk"]
                    st = tabs["stq"] if m < 2 else tabs["stk"]
                    for ip in range(NI // 2):       # isl pairs, wide tiles
                        sl = slice(ip * 1024, (ip + 1) * 1024)
                        ps = psp.tile([128, 1024], F32, name="qkvps",
                                      tag="s", bufs=2)
                        for ci in range(8):
                            for hh in range(2):
                                hs = slice((2 * ip + hh) * 512,
                                           (2 * ip + hh + 1) * 512)
                                nc.tensor.matmul(
                                    ps[:, hh * 512:(hh + 1) * 512],
                                    wts[ci][:, m * 128:(m + 1) * 128],
                                    xts[ci][:, hs],
                                    start=(ci == 0), stop=(ci == 7))
                        cps = wp.tile([128, 1024], BF, name="cps", tag="cps",
                                      bufs=2)
                        nc.vector.tensor_copy(cps[:], ps[:])
                        rot = wp.tile([128, 1024], BF, name="rot", tag="rot",
                                      bufs=2)
                        nc.vector.stream_shuffle(rot[:], cps[:], SHUF_MASK)
                        t1 = wp.tile([128, 1024], BF, name="ropet1",
                                     tag="ropet1", bufs=2)
                        nc.vector.tensor_mul(t1[:], rot[:], st[:, sl])
                        t2 = wp.tile([128, 1024], BF, name="ropet2",
                                     tag="ropet2", bufs=2)
                        nc.vector.tensor_mul(t2[:], cps[:], ct[:, sl])
                        nc.vector.tensor_add(qkp[m][:, sl], t2[:], t1[:])
                nc.leave_named_scope("qkv", sc[0], False)
                for slot in range(G):
                    src = qkp[slot // 2][(slot % 2) * 64:(slot % 2) * 64 + 64, :]
                    nc.sync.dma_start(qds[slot][0:64, :], src)
                    nc.sync.dma_start(qds[slot][64:128, :], src)

                # ---- V (non-transposed, direct; 2 j-chunks per psum) ----
                sc = nc.enter_named_scope("vdir", False)
                for jp in range(NJ // 2):
                    pv = psp.tile([128, 512], F32, name="vps",
                                  tag=f"oT{jp % 2}", bufs=1)
                    for hh in range(2):
                        j = 2 * jp + hh
                        for ci in range(8):
                            nc.tensor.matmul(
                                pv[:, hh * 256:(hh + 1) * 256],
                                xts[ci][:, j * 128:(j + 1) * 128],
                                wts[ci][:, 512:768],
                                start=(ci == 0), stop=(ci == 7))
                    vt_v = vts[jp][:].rearrange("p (a h e) -> p a h e",
                                                a=2, e=65)
                    nc.vector.tensor_copy(
                        vt_v[:, :, :, 0:64],
                        pv[:].rearrange("p (a h e) -> p a h e", a=2, e=64))
                    nc.vector.memset(vt_v[:, :, :, 64:65], 1.0)
                nc.leave_named_scope("vdir", sc[0], False)

            # ---- attention per head slot (j-outer: stationary k/v reuse) ----
            ebs = []
            ebpool = tc.tile_pool(name="ph2", bufs=1)
            p2 = ebpool.__enter__()
            for s in range(G):
                t = p2.tile([128, EB_W], BF, name=f"eb{s}")
                nc.sync.dma_start(t[:], ebias.ap()[s, :, :])
                ebs.append(t)
            ag_outs = []
            for slot in range(G):
                qh = qkp[slot // 2][(slot % 2) * 64:(slot % 2) * 64 + 64, :]
                kh = qkp[2 + slot // 2][(slot % 2) * 64:(slot % 2) * 64 + 64, :]
                cut = SLOT_CUT[slot]
                ebt = ebs[slot]
                with nc.named_scope(f"attn{slot}"):
                    # build block-diag k stationaries for all j-chunks: two
                    # partition-shifting sbuf->sbuf DMAs per slot
                    bd = bdb[slot % 2]
                    khv = kh.rearrange("p (j c) -> p j c", c=128)
                    bdv = bd[:].rearrange("p (j c) -> p j c", c=128)
                    nc.sync.dma_start(bdv[0:64, :, 0:64], khv[:, :, 0:64])
                    nc.sync.dma_start(bdv[64:128, :, 64:128], khv[:, :, 64:128])
                    kept_per_isl = [kept_j_chunks(cut, isl * 512)
                                    for isl in range(NI)]
                    oTs = [psp.tile([65, 512], F32, name=f"oT{isl}",
                                    tag=f"oT{isl}", bufs=1,
                                    padded_shape=[128, 512])
                           for isl in range(NI)]
                    for j in range(NJ):
                        j0 = j * 128
                        isls = [isl for isl in range(NI)
                                if j in kept_per_isl[isl]]
                        groups = [isls[k:k + 2] for k in range(0, len(isls), 2)]
                        ps_list = []
                        for grp in groups:
                            w = 512 * len(grp)
                            i0 = grp[0] * 512
                            s = psp.tile([128, 1024], F32, name="s", tag="s",
                                         bufs=2)
                            for hh, isl in enumerate(grp):
                                nc.tensor.matmul(
                                    s[:, hh * 512:(hh + 1) * 512],
                                    bd[:, j0:j0 + 128],
                                    qds[slot][:, isl * 512:(isl + 1) * 512],
                                    start=True, stop=True)
                            p = wp.tile([128, 1024], BF, name="p", tag="p",
                                        bufs=4)
                            nc.scalar.activation(
                                p[:, 0:w], s[:, 0:w],
                                mybir.ActivationFunctionType.Exp)
                            if j0 - (grp[-1] * 512) < 512:
                                off = i0 - j0 + EB_OFF
                                nc.vector.tensor_mul(p[:, 0:w], p[:, 0:w],
                                                     ebt[:, off:off + w])
                            ps_list.append((grp, p))
                        for grp, p in ps_list:
                            for hh, isl in enumerate(grp):
                                j_kept = kept_per_isl[isl]
                                nc.tensor.matmul(
                                    oTs[isl][:], v_ap(j, slot),
                                    p[:, hh * 512:(hh + 1) * 512],
                                    start=(j == j_kept[0]),
                                    stop=(j == j_kept[-1]))
                    for isl in range(NI):
                        i0 = isl * 512
                        oT = oTs[isl]
                        den = wp.tile([1, 512], F32, name="den", tag="den",
                                      bufs=2)
                        nc.vector.tensor_copy(den[:], oT[64:65, :])
                        rec = wp.tile([1, 512], F32, name="rec", tag="rec",
                                      bufs=2)
                        nc.vector.reciprocal_approx_fast(rec[:], den[:])
                        R = wp.tile([64, 512], F32, name="R", tag="R", bufs=2)
                        nc.gpsimd.partition_broadcast(R[:], rec[:])
                        yt = yts[slot // 2]
                        r0 = (slot % 2) * 64
                        nc.vector.tensor_mul(yt[r0:r0 + 64, i0:i0 + 512],
                                             oT[0:64, :], R[:])
                if slot % 2 == 1:
                    # AllGather the completed slot pair (within batch group)
                    half = slot // 2
                    ag_in = dp.tile([128, N], BF, name=f"ag_in{half}")
                    ag_out = dp.tile([512, N], BF, name=f"ag_out{half}")
                    ag_outs.append(ag_out)
                    nc.sync.dma_start(ag_in[:, :], yts[half][:])
                    with nc.named_scope(f"ag{half}"):
                        nc.gpsimd.collective_compute(
                            "AllGather", mybir.AluOpType.bypass,
                            replica_groups=RG4,
                            ins=[ag_in.opt()], outs=[ag_out.opt()])
            ebpool.__exit__(None, None, None)

            if dbg:
                for m in range(4):
                    t = dbg_tensor(f"dbg_qk{m}", [128, N], BF)
                    nc.sync.dma_start(t.ap()[:, :], qkp[m][:])
                t = dbg_tensor("dbg_vt0", [128, 2 * G * 65], BF)
                nc.sync.dma_start(t.ap()[:, :], vts[0][:])
                for i in range(2):
                    t = dbg_tensor(f"dbg_yt{i}", [128, N], BF)
                    nc.sync.dma_start(t.ap()[:, :], yts[i][:])

            with tc.tile_pool(name="ph3", bufs=1) as p3:
                pwts = []
                for ci in range(8):
                    t = p3.tile([128, C], BF, name=f"pwt{ci}")
                    nc.sync.dma_start(t[:], pwT.ap()[ci * 128:(ci + 1) * 128, :])
                    pwts.append(t)
                # gather my query-quarter columns of the AG'd y (indices are
                # per-core input data -> SPMD-uniform program)
                ytf = []
                for half in range(2):
                    src = ag_outs[half][:].rearrange("r (q w) -> (r q) w",
                                                     w=512)
                    for c in range(4):
                        t = p3.tile([128, 512], BF, name=f"ytf{half}_{c}")
                        nc.gpsimd.indirect_dma_start(
                            out=t[:], out_offset=None, in_=src,
                            in_offset=bass.IndirectOffsetOnAxis(
                                ap=qit[:, c:c + 1], axis=0))
                        ytf.append(t)

                # ---- projection: outT [co, my q-quarter] ----
                sc = nc.enter_named_scope("proj", False)
                for co in range(8):
                    pj = psp.tile([128, 512], F32, name="pj", tag="s", bufs=2)
                    for ci in range(8):
                        nc.tensor.matmul(pj[:],
                                         pwts[ci][:, co * 128:(co + 1) * 128],
                                         ytf[ci][:], start=(ci == 0),
                                         stop=(ci == 7))
                    ot = wp.tile([128, 512], F32, name="ot", tag="ot", bufs=2)
                    nc.scalar.add(ot[:], pj[:], pbt[:, co:co + 1])
                    nc.sync.dma_start(out.ap()[co * 128:(co + 1) * 128, :],
                                      ot[:])
                nc.leave_named_scope("proj", sc[0], False)

    nc.compile()
    return nc


def prep_inputs(x, qkv_w, proj_w, proj_b, slopes):
    """Build the 8 per-core input maps (all host-side numpy)."""
    x = np.asarray(x, np.float32)
    qkv_w = np.asarray(qkv_w, np.float32)
    proj_w = np.asarray(proj_w, np.float32)
    proj_b = np.asarray(proj_b, np.float32)
    slopes = np.asarray(slopes, np.float32)

    # RoPE tables (transposed [d, n], d pairwise-interleaved, x2 head copies)
    inv = 1.0 / (10000.0 ** (np.arange(0, D, 2, dtype=np.float32) / D))
    fr = np.arange(N, dtype=np.float32)[:, None] * inv[None, :]   # [N, 32]
    sin_t, cos_t = np.sin(fr), np.cos(fr)
    ct64 = np.empty((64, N), np.float32)
    st64 = np.empty((64, N), np.float32)
    ct64[0::2] = cos_t.T
    ct64[1::2] = cos_t.T
    st64[0::2] = -sin_t.T
    st64[1::2] = sin_t.T
    ctq = (np.vstack([ct64, ct64]) * SCALE).astype(NPBF)
    stq = (np.vstack([st64, st64]) * SCALE).astype(NPBF)
    ctk = np.vstack([ct64, ct64]).astype(NPBF)
    stk = np.vstack([st64, st64]).astype(NPBF)

    pos_p = np.arange(128, dtype=np.float64)[:, None]
    t_off = np.arange(EB_W, dtype=np.float64)[None, :] - EB_OFF
    dmin = np.minimum(pos_p - t_off, 0.0)  # j - i clipped

    in_maps = []
    for core in range(N_CORES):
        b = core // 4
        g = core % 4
        heads = GROUP_HEADS[g]
        rows = []
        for kind in range(2):  # q, k (d-permuted)
            for h in heads:
                base = kind * C + h * D
                rows.extend(base + p for p in D_PERM)
        for h in heads:        # v (natural d order)
            rows.extend(2 * C + h * D + d for d in range(D))
        wT_c = np.ascontiguousarray(qkv_w[rows, :].T)      # [1024, 768]

        eb_c = np.empty((G, 128, EB_W), np.float32)
        for s, h in enumerate(heads):
            eb_c[s] = np.exp(float(slopes[h]) * MAX_BIAS * dmin)

        # pwT rows (ci) ordered as the two AG outputs: half h rows =
        # [rank0 (slot 2h, 2h+1), rank1, ..., rank3] x 64 d each.
        pwT_c = np.empty((C, C), np.float32)
        for half in range(2):
            for rank in range(4):
                for sl2 in range(2):
                    hh = GROUP_HEADS[rank][2 * half + sl2]
                    r = 512 * half + 128 * rank + 64 * sl2
                    pwT_c[r:r + 64, :] = proj_w[:, hh * D:(hh + 1) * D].T
        pb_c = np.ascontiguousarray(proj_b.reshape(8, 128).T)

        # quarter-gather row indices into ag_out viewed as [(r q) w]
        qidx_c = np.empty((128, 4), np.uint32)
        for c in range(4):
            qidx_c[:, c] = 512 * c + 4 * np.arange(128) + g

        in_maps.append({
            "xT": np.ascontiguousarray(x[b].T).astype(NPBF),
            "wT": wT_c.astype(NPBF),
            "ctq": ctq, "stq": stq, "ctk": ctk, "stk": stk,
            "ebias": eb_c.astype(NPBF),
            "pwT": pwT_c.astype(NPBF),
            "pb": pb_c,
            "qidx": qidx_c,
        })
    return in_maps


_NC = None


def _get_nc():
    global _NC
    if _NC is None:
        _NC = build_program()
    return _NC


def run(inputs, trace=False):
    nc = _get_nc()
    in_maps = prep_inputs(**inputs)
    res = run_bass_kernel_spmd(nc, in_maps, core_ids=list(range(N_CORES)),
                               trace=trace)
    out = np.empty((B, N, C), np.float32)
    for core in range(N_CORES):
        b, g = core // 4, core % 4
        out[b, g * 512:(g + 1) * 512, :] = res.results[core]["out"].T
    return out, res


def kernel(**inputs) -> np.ndarray:
    out, _ = run(inputs, trace=False)
    return out


# revision 37
# speedup vs baseline: 2.2040x; 1.0510x over previous
"""Trainium2 Bass kernel: attention block (QKV + RoPE + ALiBi attention + proj).

Problem shapes: x [2, 2048, 1024], 16 heads x 64 dim, f32 I/O.
Sharding: batch (2) x head-groups (4 heads/core) = 8 cores. Two 4-core
AllGathers exchange per-head outputs; each core then gathers its own
query-quarter columns via indirect DMA (per-core indices are input data,
keeping the SPMD program uniform) and runs the output projection.

Dataflow is fully transposed on-chip:
  xT [C, N] --QKV--> qT/kT [D, N] (RoPE'd, d pairwise-interleaved) and v [N, D]
  sT = kT_chunk.T @ qT   [j, i]  -> p = exp(sT) * ebias_table  (no max-sub)
  oT = v_aug.T @ p       [d+1, i] (ones column gives softmax denominators)
  yT = oT[:64] * (1/denom broadcast) -> AG -> outT = pwT.T @ yT[:, my q] + b
Host transposes final outT back. All matmuls bf16 with f32 PSUM accumulation.
ALiBi is applied multiplicatively (exp(slope*min(j-i,0)) Toeplitz tables) and
far-past tiles below ~1e-3 relative mass are skipped per-slot (SLOT_CUT).
"""
import sys
if "/opt/trn_rl_repo" not in sys.path:
    sys.path.insert(0, "/opt/trn_rl_repo")

import numpy as np
import ml_dtypes

import concourse.bass as bass
import concourse.mybir as mybir
import concourse.tile as tile
from concourse import bacc
from concourse.bass_utils import run_bass_kernel_spmd

BF = mybir.dt.bfloat16
F32 = mybir.dt.float32
NPBF = ml_dtypes.bfloat16

B, N, C = 2, 2048, 1024
H, D = 16, 64
G = 4                       # heads per core
N_CORES = 8
MAX_BIAS = 8.0
SCALE = D ** -0.5
RG4 = [[0, 1, 2, 3], [4, 5, 6, 7]]

# head dealing (snake by per-head band cost) and per-slot band cutoffs
# (validated empirically: truncation error << bf16 error).
GROUP_HEADS = [[15, 8, 7, 0], [14, 9, 6, 1], [13, 10, 5, 2], [12, 11, 4, 3]]
SLOT_CUT = [384, 192, 128, 128]

NJ = N // 128               # 16 j-chunks
NI = N // 512               # 4 i-slices
EB_W = 2048                 # ebias table width; u = (i - j) + EB_OFF
EB_OFF = 1023

# d-permutation inside each head: [0, 32, 1, 33, ...] so rotate_half becomes
# an even/odd partition swap (StreamShuffle-able within 32-row quadrants).
D_PERM = [x for i in range(32) for x in (i, i + 32)]
SHUF_MASK = [x for i in range(16) for x in (2 * i + 1, 2 * i)]


def kept_j_chunks(cut, i0):
    return [j for j in range(NJ) if i0 - (j * 128 + 127) <= cut]


def build_program(dbg=False):
    nc = bacc.Bacc("TRN2", target_bir_lowering=False, debug=False,
                   num_devices=N_CORES)
    dbg_outs = {}

    def dbg_tensor(name, shape, dt_=F32):
        dbg_outs[name] = nc.dram_tensor(name, shape, dt_, kind="ExternalOutput")
        return dbg_outs[name]

    xT = nc.dram_tensor("xT", [C, N], BF, kind="ExternalInput")
    wT = nc.dram_tensor("wT", [C, 768], BF, kind="ExternalInput")
    ctq = nc.dram_tensor("ctq", [128, N], BF, kind="ExternalInput")
    stq = nc.dram_tensor("stq", [128, N], BF, kind="ExternalInput")
    ctk = nc.dram_tensor("ctk", [128, N], BF, kind="ExternalInput")
    stk = nc.dram_tensor("stk", [128, N], BF, kind="ExternalInput")
    ebias = nc.dram_tensor("ebias", [G, 128, EB_W], BF, kind="ExternalInput")
    pwT = nc.dram_tensor("pwT", [C, C], BF, kind="ExternalInput")
    pb = nc.dram_tensor("pb", [128, 8], F32, kind="ExternalInput")
    qidx = nc.dram_tensor("qidx", [128, 2], mybir.dt.uint32,
                          kind="ExternalInput")
    out = nc.dram_tensor("out", [C, 512], F32, kind="ExternalOutput")

    with tile.TileContext(nc) as tc:
        with tc.tile_pool(name="persist", bufs=1) as pp, \
             tc.tile_pool(name="work", bufs=1) as wp, \
             tc.tile_pool(name="psum", bufs=1, space="PSUM") as psp, \
             tc.tile_pool(name="dram", bufs=1, space="DRAM") as dp:

            # ---- persistent tiles (live across phases) ----
            qkp = [pp.tile([128, N], BF, name=f"qk{m}") for m in range(4)]
            vts = [pp.tile([128, 2 * G * 65], BF, name=f"vt{j}")
                   for j in range(NJ // 2)]

            def v_ap(j, slot):
                # stationary v_aug [128, 65] for (j-chunk, slot)
                base = (j % 2) * G * 65 + slot * 65
                return vts[j // 2][:, base:base + 65]

            yts = [pp.tile([128, N], BF, name=f"yt{i}") for i in range(2)]
            pbt = pp.tile([128, 8], F32, name="pbt")
            nc.sync.dma_start(pbt[:], pb.ap()[:, :])
            qit = pp.tile([128, 2], mybir.dt.uint32, name="qit")
            nc.sync.dma_start(qit[:], qidx.ap()[:, :])
            # per-slot duplicated q (so QK's moving operand spans 128
            # partitions) and block-diag k stationaries (two 64-row j-chunks
            # of one head on the diagonal -> K=128 full-rate streaming)
            qds = [pp.tile([128, N], BF, name=f"qd{s}") for s in range(G)]
            bdb = [pp.tile([128, N], BF, name=f"bd{i}") for i in range(2)]
            nc.vector.memset(bdb[0][:], 0.0)
            nc.vector.memset(bdb[1][:], 0.0)

            with tc.tile_pool(name="ph1", bufs=1) as p1:
                # ---- phase-1 inputs ----
                xts = []
                for ci in range(8):
                    t = p1.tile([128, N], BF, name=f"xt{ci}")
                    nc.sync.dma_start(t[:], xT.ap()[ci * 128:(ci + 1) * 128, :])
                    xts.append(t)
                wts = []
                for ci in range(8):
                    t = p1.tile([128, 768], BF, name=f"wt{ci}")
                    nc.sync.dma_start(t[:], wT.ap()[ci * 128:(ci + 1) * 128, :])
                    wts.append(t)
                tabs = {}
                for nm, dt_ in (("ctq", ctq), ("stq", stq), ("ctk", ctk),
                                ("stk", stk)):
                    t = p1.tile([128, N], BF, name=f"tab_{nm}")
                    nc.sync.dma_start(t[:], dt_.ap()[:, :])
                    tabs[nm] = t

                # ---- QKV (q/k) + RoPE ----
                # wT cols: q01 | q23 | k01 | k23 | v (4x64)
                sc = nc.enter_named_scope("qkv", False)
                for m in range(4):
                    ct = tabs["ctq"] if m < 2 else tabs["ctk"]
                    st = tabs["stq"] if m < 2 else tabs["stk"]
                    for ip in range(NI // 2):       # isl pairs, wide tiles
                        sl = slice(ip * 1024, (ip + 1) * 1024)
                        ps = psp.tile([128, 1024], F32, name="qkvps",
                                      tag="s", bufs=2)
                        for ci in range(8):
                            for hh in range(2):
                                hs = slice((2 * ip + hh) * 512,
                                           (2 * ip + hh + 1) * 512)
                                nc.tensor.matmul(
                                    ps[:, hh * 512:(hh + 1) * 512],
                                    wts[ci][:, m * 128:(m + 1) * 128],
                                    xts[ci][:, hs],
                                    start=(ci == 0), stop=(ci == 7))
                        cps = wp.tile([128, 1024], BF, name="cps", tag="cps",
                                      bufs=2)
                        nc.vector.tensor_copy(cps[:], ps[:])
                        rot = wp.tile([128, 1024], BF, name="rot", tag="rot",
                                      bufs=2)
                        nc.vector.stream_shuffle(rot[:], cps[:], SHUF_MASK)
                        t1 = wp.tile([128, 1024], BF, name="ropet1",
                                     tag="ropet1", bufs=2)
                        nc.vector.tensor_mul(t1[:], rot[:], st[:, sl])
                        t2 = wp.tile([128, 1024], BF, name="ropet2",
                                     tag="ropet2", bufs=2)
                        nc.vector.tensor_mul(t2[:], cps[:], ct[:, sl])
                        nc.vector.tensor_add(qkp[m][:, sl], t2[:], t1[:])
                nc.leave_named_scope("qkv", sc[0], False)
                for slot in range(G):
                    src = qkp[slot // 2][(slot % 2) * 64:(slot % 2) * 64 + 64, :]
                    nc.sync.dma_start(qds[slot][0:64, :], src)
                    nc.sync.dma_start(qds[slot][64:128, :], src)

                # ---- V (non-transposed, direct; 2 j-chunks per psum) ----
                sc = nc.enter_named_scope("vdir", False)
                for jp in range(NJ // 2):
                    pv = psp.tile([128, 512], F32, name="vps",
                                  tag=f"oT{jp % 2}", bufs=1)
                    for hh in range(2):
                        j = 2 * jp + hh
                        for ci in range(8):
                            nc.tensor.matmul(
                                pv[:, hh * 256:(hh + 1) * 256],
                                xts[ci][:, j * 128:(j + 1) * 128],
                                wts[ci][:, 512:768],
                                start=(ci == 0), stop=(ci == 7))
                    vt_v = vts[jp][:].rearrange("p (a h e) -> p a h e",
                                                a=2, e=65)
                    nc.vector.tensor_copy(
                        vt_v[:, :, :, 0:64],
                        pv[:].rearrange("p (a h e) -> p a h e", a=2, e=64))
                    nc.vector.memset(vt_v[:, :, :, 64:65], 1.0)
                nc.leave_named_scope("vdir", sc[0], False)

            # ---- attention per head slot (j-outer: stationary k/v reuse) ----
            ebs = []
            ebpool = tc.tile_pool(name="ph2", bufs=1)
            p2 = ebpool.__enter__()
            for s in range(G):
                t = p2.tile([128, EB_W], BF, name=f"eb{s}")
                nc.sync.dma_start(t[:], ebias.ap()[s, :, :])
                ebs.append(t)
            ag_outs = []
            for slot in range(G):
                qh = qkp[slot // 2][(slot % 2) * 64:(slot % 2) * 64 + 64, :]
                kh = qkp[2 + slot // 2][(slot % 2) * 64:(slot % 2) * 64 + 64, :]
                cut = SLOT_CUT[slot]
                ebt = ebs[slot]
                with nc.named_scope(f"attn{slot}"):
                    # build block-diag k stationaries for all j-chunks: two
                    # partition-shifting sbuf->sbuf DMAs per slot
                    bd = bdb[slot % 2]
                    khv = kh.rearrange("p (j c) -> p j c", c=128)
                    bdv = bd[:].rearrange("p (j c) -> p j c", c=128)
                    nc.sync.dma_start(bdv[0:64, :, 0:64], khv[:, :, 0:64])
                    nc.sync.dma_start(bdv[64:128, :, 64:128], khv[:, :, 64:128])
                    kept_per_isl = [kept_j_chunks(cut, isl * 512)
                                    for isl in range(NI)]
                    oTs = [psp.tile([65, 512], F32, name=f"oT{isl}",
                                    tag=f"oT{isl}", bufs=1,
                                    padded_shape=[128, 512])
                           for isl in range(NI)]
                    for j in range(NJ):
                        j0 = j * 128
                        isls = [isl for isl in range(NI)
                                if j in kept_per_isl[isl]]
                        groups = [isls[k:k + 2] for k in range(0, len(isls), 2)]
                        ps_list = []
                        for grp in groups:
                            w = 512 * len(grp)
                            i0 = grp[0] * 512
                            s = psp.tile([128, 1024], F32, name="s", tag="s",
                                         bufs=2)
                            for hh, isl in enumerate(grp):
                                nc.tensor.matmul(
                                    s[:, hh * 512:(hh + 1) * 512],
                                    bd[:, j0:j0 + 128],
                                    qds[slot][:, isl * 512:(isl + 1) * 512],
                                    start=True, stop=True)
                            p = wp.tile([128, 1024], BF, name="p", tag="p",
                                        bufs=4)
                            nc.scalar.activation(
                                p[:, 0:w], s[:, 0:w],
                                mybir.ActivationFunctionType.Exp)
                            if j0 - (grp[-1] * 512) < 512:
                                off = i0 - j0 + EB_OFF
                                nc.vector.tensor_mul(p[:, 0:w], p[:, 0:w],
                                                     ebt[:, off:off + w])
                            ps_list.append((grp, p))
                        for grp, p in ps_list:
                            for hh, isl in enumerate(grp):
                                j_kept = kept_per_isl[isl]
                                nc.tensor.matmul(
                                    oTs[isl][:], v_ap(j, slot),
                                    p[:, hh * 512:(hh + 1) * 512],
                                    start=(j == j_kept[0]),
                                    stop=(j == j_kept[-1]))
                    for isl in range(NI):
                        i0 = isl * 512
                        oT = oTs[isl]
                        den = wp.tile([1, 512], F32, name="den", tag="den",
                                      bufs=2)
                        nc.vector.tensor_copy(den[:], oT[64:65, :])
                        rec = wp.tile([1, 512], F32, name="rec", tag="rec",
                                      bufs=2)
                        nc.vector.reciprocal_approx_fast(rec[:], den[:])
                        R = wp.tile([64, 512], F32, name="R", tag="R", bufs=2)
                        nc.gpsimd.partition_broadcast(R[:], rec[:])
                        yt = yts[slot // 2]
                        r0 = (slot % 2) * 64
                        nc.vector.tensor_mul(yt[r0:r0 + 64, i0:i0 + 512],
                                             oT[0:64, :], R[:])
                # AllGather this slot's y (within batch group)
                ag_in = dp.tile([64, N], BF, name=f"ag_in{slot}")
                ag_out = dp.tile([256, N], BF, name=f"ag_out{slot}")
                ag_outs.append(ag_out)
                r0 = (slot % 2) * 64
                nc.sync.dma_start(ag_in[:, :], yts[slot // 2][r0:r0 + 64, :])
                with nc.named_scope(f"ag{slot}"):
                    nc.gpsimd.collective_compute(
                        "AllGather", mybir.AluOpType.bypass,
                        replica_groups=RG4,
                        ins=[ag_in.opt()], outs=[ag_out.opt()])
            ebpool.__exit__(None, None, None)

            if dbg:
                for m in range(4):
                    t = dbg_tensor(f"dbg_qk{m}", [128, N], BF)
                    nc.sync.dma_start(t.ap()[:, :], qkp[m][:])
                t = dbg_tensor("dbg_vt0", [128, 2 * G * 65], BF)
                nc.sync.dma_start(t.ap()[:, :], vts[0][:])
                for i in range(2):
                    t = dbg_tensor(f"dbg_yt{i}", [128, N], BF)
                    nc.sync.dma_start(t.ap()[:, :], yts[i][:])

            with tc.tile_pool(name="ph3", bufs=1) as p3:
                pwts = []
                for ci in range(8):
                    t = p3.tile([128, C], BF, name=f"pwt{ci}")
                    nc.sync.dma_start(t[:], pwT.ap()[ci * 128:(ci + 1) * 128, :])
                    pwts.append(t)
                # gather my query-quarter columns of the AG'd y (indices are
                # per-core input data -> SPMD-uniform program)
                ytf = []
                for s in range(4):
                    srcv = ag_outs[s][:].rearrange("r (q w) -> (r q) w", w=512)
                    for hh in range(2):
                        t = p3.tile([128, 512], BF, name=f"ytf{s}_{hh}")
                        nc.gpsimd.indirect_dma_start(
                            out=t[:], out_offset=None, in_=srcv,
                            in_offset=bass.IndirectOffsetOnAxis(
                                ap=qit[:, hh:hh + 1], axis=0))
                        ytf.append(t)

                # ---- projection: outT [co, my q-quarter] ----
                sc = nc.enter_named_scope("proj", False)
                for co in range(8):
                    pj = psp.tile([128, 512], F32, name="pj", tag="s", bufs=2)
                    for ci in range(8):
                        nc.tensor.matmul(pj[:],
                                         pwts[ci][:, co * 128:(co + 1) * 128],
                                         ytf[ci][:], start=(ci == 0),
                                         stop=(ci == 7))
                    ot = wp.tile([128, 512], F32, name="ot", tag="ot", bufs=2)
                    nc.scalar.add(ot[:], pj[:], pbt[:, co:co + 1])
                    nc.sync.dma_start(out.ap()[co * 128:(co + 1) * 128, :],
                                      ot[:])
                nc.leave_named_scope("proj", sc[0], False)

    nc.compile()
    return nc


def prep_inputs(x, qkv_w, proj_w, proj_b, slopes):
    """Build the 8 per-core input maps (all host-side numpy)."""
    x = np.asarray(x, np.float32)
    qkv_w = np.asarray(qkv_w, np.float32)
    proj_w = np.asarray(proj_w, np.float32)
    proj_b = np.asarray(proj_b, np.float32)
    slopes = np.asarray(slopes, np.float32)

    # RoPE tables (transposed [d, n], d pairwise-interleaved, x2 head copies)
    inv = 1.0 / (10000.0 ** (np.arange(0, D, 2, dtype=np.float32) / D))
    fr = np.arange(N, dtype=np.float32)[:, None] * inv[None, :]   # [N, 32]
    sin_t, cos_t = np.sin(fr), np.cos(fr)
    ct64 = np.empty((64, N), np.float32)
    st64 = np.empty((64, N), np.float32)
    ct64[0::2] = cos_t.T
    ct64[1::2] = cos_t.T
    st64[0::2] = -sin_t.T
    st64[1::2] = sin_t.T
    ctq = (np.vstack([ct64, ct64]) * SCALE).astype(NPBF)
    stq = (np.vstack([st64, st64]) * SCALE).astype(NPBF)
    ctk = np.vstack([ct64, ct64]).astype(NPBF)
    stk = np.vstack([st64, st64]).astype(NPBF)

    pos_p = np.arange(128, dtype=np.float64)[:, None]
    t_off = np.arange(EB_W, dtype=np.float64)[None, :] - EB_OFF
    dmin = np.minimum(pos_p - t_off, 0.0)  # j - i clipped

    in_maps = []
    for core in range(N_CORES):
        b = core // 4
        g = core % 4
        heads = GROUP_HEADS[g]
        rows = []
        for kind in range(2):  # q, k (d-permuted)
            for h in heads:
                base = kind * C + h * D
                rows.extend(base + p for p in D_PERM)
        for h in heads:        # v (natural d order)
            rows.extend(2 * C + h * D + d for d in range(D))
        wT_c = np.ascontiguousarray(qkv_w[rows, :].T)      # [1024, 768]

        eb_c = np.empty((G, 128, EB_W), np.float32)
        for s, h in enumerate(heads):
            eb_c[s] = np.exp(float(slopes[h]) * MAX_BIAS * dmin)

        # pwT rows (ci) ordered as the two AG outputs: half h rows =
        # [rank0 (slot 2h, 2h+1), rank1, ..., rank3] x 64 d each.
        pwT_c = np.empty((C, C), np.float32)
        for s in range(4):
            for rank in range(4):
                hh = GROUP_HEADS[rank][s]
                r = 256 * s + 64 * rank
                pwT_c[r:r + 64, :] = proj_w[:, hh * D:(hh + 1) * D].T
        pb_c = np.ascontiguousarray(proj_b.reshape(8, 128).T)

        # quarter-gather row indices into ag_out viewed as [(r q) w]
        qidx_c = np.empty((128, 2), np.uint32)
        for c in range(2):
            qidx_c[:, c] = (128 * c + np.arange(128)) * 4 + g

        in_maps.append({
            "xT": np.ascontiguousarray(x[b].T).astype(NPBF),
            "wT": wT_c.astype(NPBF),
            "ctq": ctq, "stq": stq, "ctk": ctk, "stk": stk,
            "ebias": eb_c.astype(NPBF),
            "pwT": pwT_c.astype(NPBF),
            "pb": pb_c,
            "qidx": qidx_c,
        })
    return in_maps


_NC = None


def _get_nc():
    global _NC
    if _NC is None:
        _NC = build_program()
    return _NC


def run(inputs, trace=False):
    nc = _get_nc()
    in_maps = prep_inputs(**inputs)
    res = run_bass_kernel_spmd(nc, in_maps, core_ids=list(range(N_CORES)),
                               trace=trace)
    out = np.empty((B, N, C), np.float32)
    for core in range(N_CORES):
        b, g = core // 4, core % 4
        out[b, g * 512:(g + 1) * 512, :] = res.results[core]["out"].T
    return out, res


def kernel(**inputs) -> np.ndarray:
    out, _ = run(inputs, trace=False)
    return out


# revision 39
# speedup vs baseline: 2.2758x; 1.0326x over previous
"""Trainium2 Bass kernel: attention block (QKV + RoPE + ALiBi attention + proj).

Problem shapes: x [2, 2048, 1024], 16 heads x 64 dim, f32 I/O.
Sharding: batch (2) x head-groups (4 heads/core) = 8 cores. Two 4-core
AllGathers exchange per-head outputs; each core then gathers its own
query-quarter columns via indirect DMA (per-core indices are input data,
keeping the SPMD program uniform) and runs the output projection.

Dataflow is fully transposed on-chip:
  xT [C, N] --QKV--> qT/kT [D, N] (RoPE'd, d pairwise-interleaved) and v [N, D]
  sT = kT_chunk.T @ qT   [j, i]  -> p = exp(sT) * ebias_table  (no max-sub)
  oT = v_aug.T @ p       [d+1, i] (ones column gives softmax denominators)
  yT = oT[:64] * (1/denom broadcast) -> AG -> outT = pwT.T @ yT[:, my q] + b
Host transposes final outT back. All matmuls bf16 with f32 PSUM accumulation.
ALiBi is applied multiplicatively (exp(slope*min(j-i,0)) Toeplitz tables) and
far-past tiles below ~1e-3 relative mass are skipped per-slot (SLOT_CUT).
"""
import sys
if "/opt/trn_rl_repo" not in sys.path:
    sys.path.insert(0, "/opt/trn_rl_repo")

import numpy as np
import ml_dtypes

import concourse.bass as bass
import concourse.mybir as mybir
import concourse.tile as tile
from concourse import bacc
from concourse.bass_utils import run_bass_kernel_spmd

BF = mybir.dt.bfloat16
F32 = mybir.dt.float32
NPBF = ml_dtypes.bfloat16

B, N, C = 2, 2048, 1024
H, D = 16, 64
G = 4                       # heads per core
N_CORES = 8
MAX_BIAS = 8.0
SCALE = D ** -0.5
RG4 = [[0, 1, 2, 3], [4, 5, 6, 7]]
RG8 = [[0, 1, 2, 3, 4, 5, 6, 7]]

# head dealing (snake by per-head band cost) and per-slot band cutoffs
# (validated empirically: truncation error << bf16 error).
GROUP_HEADS = [[15, 8, 7, 0], [14, 9, 6, 1], [13, 10, 5, 2], [12, 11, 4, 3]]
SLOT_CUT = [384, 192, 128, 128]

NJ = N // 128               # 16 j-chunks
NI = N // 512               # 4 i-slices
EB_W = 2048                 # ebias table width; u = (i - j) + EB_OFF
EB_OFF = 1023

# d-permutation inside each head: [0, 32, 1, 33, ...] so rotate_half becomes
# an even/odd partition swap (StreamShuffle-able within 32-row quadrants).
D_PERM = [x for i in range(32) for x in (i, i + 32)]
SHUF_MASK = [x for i in range(16) for x in (2 * i + 1, 2 * i)]


def kept_j_chunks(cut, i0):
    return [j for j in range(NJ) if i0 - (j * 128 + 127) <= cut]


def build_program(dbg=False):
    nc = bacc.Bacc("TRN2", target_bir_lowering=False, debug=False,
                   num_devices=N_CORES)
    dbg_outs = {}

    def dbg_tensor(name, shape, dt_=F32):
        dbg_outs[name] = nc.dram_tensor(name, shape, dt_, kind="ExternalOutput")
        return dbg_outs[name]

    xT = nc.dram_tensor("xT", [C, N], BF, kind="ExternalInput")
    wT = nc.dram_tensor("wT", [C, 768], BF, kind="ExternalInput")
    ctq = nc.dram_tensor("ctq", [128, N], BF, kind="ExternalInput")
    stq = nc.dram_tensor("stq", [128, N], BF, kind="ExternalInput")
    ctk = nc.dram_tensor("ctk", [128, N], BF, kind="ExternalInput")
    stk = nc.dram_tensor("stk", [128, N], BF, kind="ExternalInput")
    ebias = nc.dram_tensor("ebias", [G, 128, EB_W], BF, kind="ExternalInput")
    pwT = nc.dram_tensor("pwT", [C, C], BF, kind="ExternalInput")
    pb = nc.dram_tensor("pb", [128, 8], F32, kind="ExternalInput")
    qidx = nc.dram_tensor("qidx", [128, 2], mybir.dt.uint32,
                          kind="ExternalInput")
    out = nc.dram_tensor("out", [C, 512], F32, kind="ExternalOutput")

    with tile.TileContext(nc) as tc:
        with tc.tile_pool(name="persist", bufs=1) as pp, \
             tc.tile_pool(name="work", bufs=1) as wp, \
             tc.tile_pool(name="psum", bufs=1, space="PSUM") as psp, \
             tc.tile_pool(name="dram", bufs=1, space="DRAM") as dp:

            # ---- persistent tiles (live across phases) ----
            qkp = [pp.tile([128, N], BF, name=f"qk{m}") for m in range(4)]
            vts = [pp.tile([128, 2 * G * 65], BF, name=f"vt{j}")
                   for j in range(NJ // 2)]

            def v_ap(j, slot):
                # stationary v_aug [128, 65] for (j-chunk, slot)
                base = (j % 2) * G * 65 + slot * 65
                return vts[j // 2][:, base:base + 65]

            yts = [pp.tile([128, N], BF, name=f"yt{i}") for i in range(2)]
            pbt = pp.tile([128, 8], F32, name="pbt")
            nc.sync.dma_start(pbt[:], pb.ap()[:, :])
            qit = pp.tile([128, 2], mybir.dt.uint32, name="qit")
            nc.sync.dma_start(qit[:], qidx.ap()[:, :])
            # per-slot duplicated q (so QK's moving operand spans 128
            # partitions) and block-diag k stationaries (two 64-row j-chunks
            # of one head on the diagonal -> K=128 full-rate streaming)
            qds = [pp.tile([128, N], BF, name=f"qd{s}") for s in range(G)]
            bdb = [pp.tile([128, N], BF, name=f"bd{i}") for i in range(2)]
            nc.vector.memset(bdb[0][:], 0.0)
            nc.vector.memset(bdb[1][:], 0.0)

            with tc.tile_pool(name="ph1", bufs=1) as p1:
                # ---- phase-1 inputs ----
                xts = []
                for ci in range(8):
                    t = p1.tile([128, N], BF, name=f"xt{ci}")
                    nc.sync.dma_start(t[:], xT.ap()[ci * 128:(ci + 1) * 128, :])
                    xts.append(t)
                wts = []
                for ci in range(8):
                    t = p1.tile([128, 768], BF, name=f"wt{ci}")
                    nc.sync.dma_start(t[:], wT.ap()[ci * 128:(ci + 1) * 128, :])
                    wts.append(t)
                tabs = {}
                for nm, dt_ in (("ctq", ctq), ("stq", stq), ("ctk", ctk),
                                ("stk", stk)):
                    t = p1.tile([128, N], BF, name=f"tab_{nm}")
                    nc.sync.dma_start(t[:], dt_.ap()[:, :])
                    tabs[nm] = t

                # ---- QKV (q/k) + RoPE ----
                # wT cols: q01 | q23 | k01 | k23 | v (4x64)
                sc = nc.enter_named_scope("qkv", False)
                for m in range(4):
                    ct = tabs["ctq"] if m < 2 else tabs["ctk"]
                    st = tabs["stq"] if m < 2 else tabs["stk"]
                    for ip in range(NI // 2):       # isl pairs, wide tiles
                        sl = slice(ip * 1024, (ip + 1) * 1024)
                        ps = psp.tile([128, 1024], F32, name="qkvps",
                                      tag="s", bufs=2)
                        for ci in range(8):
                            for hh in range(2):
                                hs = slice((2 * ip + hh) * 512,
                                           (2 * ip + hh + 1) * 512)
                                nc.tensor.matmul(
                                    ps[:, hh * 512:(hh + 1) * 512],
                                    wts[ci][:, m * 128:(m + 1) * 128],
                                    xts[ci][:, hs],
                                    start=(ci == 0), stop=(ci == 7))
                        cps = wp.tile([128, 1024], BF, name="cps", tag="cps",
                                      bufs=2)
                        nc.scalar.copy(cps[:], ps[:])
                        rot = wp.tile([128, 1024], BF, name="rot", tag="rot",
                                      bufs=2)
                        nc.vector.stream_shuffle(rot[:], cps[:], SHUF_MASK)
                        t1 = wp.tile([128, 1024], BF, name="ropet1",
                                     tag="ropet1", bufs=2)
                        nc.vector.tensor_mul(t1[:], rot[:], st[:, sl])
                        t2 = wp.tile([128, 1024], BF, name="ropet2",
                                     tag="ropet2", bufs=2)
                        nc.vector.tensor_mul(t2[:], cps[:], ct[:, sl])
                        nc.gpsimd.tensor_add(qkp[m][:, sl], t2[:], t1[:])
                nc.leave_named_scope("qkv", sc[0], False)
                for slot in range(G):
                    src = qkp[slot // 2][(slot % 2) * 64:(slot % 2) * 64 + 64, :]
                    nc.sync.dma_start(qds[slot][0:64, :], src)
                    nc.sync.dma_start(qds[slot][64:128, :], src)

                # ---- V (non-transposed, direct; 2 j-chunks per psum) ----
                sc = nc.enter_named_scope("vdir", False)
                for jp in range(NJ // 2):
                    pv = psp.tile([128, 512], F32, name="vps",
                                  tag=f"oT{jp % 2}", bufs=1)
                    for hh in range(2):
                        j = 2 * jp + hh
                        for ci in range(8):
                            nc.tensor.matmul(
                                pv[:, hh * 256:(hh + 1) * 256],
                                xts[ci][:, j * 128:(j + 1) * 128],
                                wts[ci][:, 512:768],
                                start=(ci == 0), stop=(ci == 7))
                    vt_v = vts[jp][:].rearrange("p (a h e) -> p a h e",
                                                a=2, e=65)
                    nc.vector.tensor_copy(
                        vt_v[:, :, :, 0:64],
                        pv[:].rearrange("p (a h e) -> p a h e", a=2, e=64))
                    nc.vector.memset(vt_v[:, :, :, 64:65], 1.0)
                nc.leave_named_scope("vdir", sc[0], False)

            # ---- attention per head slot (j-outer: stationary k/v reuse) ----
            ebs = []
            ebpool = tc.tile_pool(name="ph2", bufs=1)
            p2 = ebpool.__enter__()
            for s in range(G):
                t = p2.tile([128, EB_W], BF, name=f"eb{s}")
                nc.sync.dma_start(t[:], ebias.ap()[s, :, :])
                ebs.append(t)
            ag_outs = []
            for slot in range(G):
                qh = qkp[slot // 2][(slot % 2) * 64:(slot % 2) * 64 + 64, :]
                kh = qkp[2 + slot // 2][(slot % 2) * 64:(slot % 2) * 64 + 64, :]
                cut = SLOT_CUT[slot]
                ebt = ebs[slot]
                with nc.named_scope(f"attn{slot}"):
                    # build block-diag k stationaries for all j-chunks: two
                    # partition-shifting sbuf->sbuf DMAs per slot
                    bd = bdb[slot % 2]
                    khv = kh.rearrange("p (j c) -> p j c", c=128)
                    bdv = bd[:].rearrange("p (j c) -> p j c", c=128)
                    nc.sync.dma_start(bdv[0:64, :, 0:64], khv[:, :, 0:64])
                    nc.sync.dma_start(bdv[64:128, :, 64:128], khv[:, :, 64:128])
                    kept_per_isl = [kept_j_chunks(cut, isl * 512)
                                    for isl in range(NI)]
                    oTs = [psp.tile([65, 512], F32, name=f"oT{isl}",
                                    tag=f"oT{isl}", bufs=1,
                                    padded_shape=[128, 512])
                           for isl in range(NI)]
                    for j in range(NJ):
                        j0 = j * 128
                        isls = [isl for isl in range(NI)
                                if j in kept_per_isl[isl]]
                        groups = [isls[k:k + 2] for k in range(0, len(isls), 2)]
                        ps_list = []
                        for grp in groups:
                            w = 512 * len(grp)
                            i0 = grp[0] * 512
                            s = psp.tile([128, 1024], F32, name="s", tag="s",
                                         bufs=2)
                            for hh, isl in enumerate(grp):
                                nc.tensor.matmul(
                                    s[:, hh * 512:(hh + 1) * 512],
                                    bd[:, j0:j0 + 128],
                                    qds[slot][:, isl * 512:(isl + 1) * 512],
                                    start=True, stop=True)
                            p = wp.tile([128, 1024], BF, name="p", tag="p",
                                        bufs=6)
                            nc.scalar.activation(
                                p[:, 0:w], s[:, 0:w],
                                mybir.ActivationFunctionType.Exp)
                            if j0 - (grp[-1] * 512) < 512:
                                off = i0 - j0 + EB_OFF
                                nc.vector.tensor_mul(p[:, 0:w], p[:, 0:w],
                                                     ebt[:, off:off + w])
                            ps_list.append((grp, p))
                        for grp, p in ps_list:
                            for hh, isl in enumerate(grp):
                                j_kept = kept_per_isl[isl]
                                nc.tensor.matmul(
                                    oTs[isl][:], v_ap(j, slot),
                                    p[:, hh * 512:(hh + 1) * 512],
                                    start=(j == j_kept[0]),
                                    stop=(j == j_kept[-1]))
                    for isl in range(NI):
                        i0 = isl * 512
                        oT = oTs[isl]
                        den = wp.tile([1, 512], F32, name="den", tag="den",
                                      bufs=2)
                        nc.vector.tensor_copy(den[:], oT[64:65, :])
                        rec = wp.tile([1, 512], F32, name="rec", tag="rec",
                                      bufs=2)
                        nc.vector.reciprocal_approx_fast(rec[:], den[:])
                        R = wp.tile([64, 512], F32, name="R", tag="R", bufs=2)
                        nc.gpsimd.partition_broadcast(R[:], rec[:])
                        yt = yts[slot // 2]
                        r0 = (slot % 2) * 64
                        nc.vector.tensor_mul(yt[r0:r0 + 64, i0:i0 + 512],
                                             oT[0:64, :], R[:])
                # AllToAll this slot's y: shard j = (slot-y, quarter j%4)
                ag_in = dp.tile([512, 512], BF, name=f"ag_in{slot}")
                ag_out = dp.tile([512, 512], BF, name=f"ag_out{slot}")
                ag_outs.append(ag_out)
                r0 = (slot % 2) * 64
                for shard in range(8):
                    nc.sync.dma_start(
                        ag_in[shard * 64:(shard + 1) * 64, :],
                        yts[slot // 2][r0:r0 + 64,
                                       (shard % 4) * 512:(shard % 4 + 1) * 512])
                with nc.named_scope(f"a2a{slot}"):
                    nc.gpsimd.collective_compute(
                        "AllToAll", mybir.AluOpType.bypass,
                        replica_groups=RG8,
                        ins=[ag_in.opt()], outs=[ag_out.opt()])
            ebpool.__exit__(None, None, None)

            if dbg:
                for m in range(4):
                    t = dbg_tensor(f"dbg_qk{m}", [128, N], BF)
                    nc.sync.dma_start(t.ap()[:, :], qkp[m][:])
                t = dbg_tensor("dbg_vt0", [128, 2 * G * 65], BF)
                nc.sync.dma_start(t.ap()[:, :], vts[0][:])
                for i in range(2):
                    t = dbg_tensor(f"dbg_yt{i}", [128, N], BF)
                    nc.sync.dma_start(t.ap()[:, :], yts[i][:])

            with tc.tile_pool(name="ph3", bufs=1) as p3:
                pwts = []
                for ci in range(8):
                    t = p3.tile([128, C], BF, name=f"pwt{ci}")
                    nc.sync.dma_start(t[:], pwT.ap()[ci * 128:(ci + 1) * 128, :])
                    pwts.append(t)
                # gather my query-quarter columns of the AG'd y (indices are
                # per-core input data -> SPMD-uniform program)
                ytf = []
                for s in range(4):
                    for hh in range(2):
                        t = p3.tile([128, 512], BF, name=f"ytf{s}_{hh}")
                        nc.gpsimd.indirect_dma_start(
                            out=t[:], out_offset=None, in_=ag_outs[s][:],
                            in_offset=bass.IndirectOffsetOnAxis(
                                ap=qit[:, hh:hh + 1], axis=0))
                        ytf.append(t)

                # ---- projection: outT [co, my q-quarter] ----
                sc = nc.enter_named_scope("proj", False)
                for co in range(8):
                    pj = psp.tile([128, 512], F32, name="pj", tag="s", bufs=2)
                    for ci in range(8):
                        nc.tensor.matmul(pj[:],
                                         pwts[ci][:, co * 128:(co + 1) * 128],
                                         ytf[ci][:], start=(ci == 0),
                                         stop=(ci == 7))
                    ot = wp.tile([128, 512], F32, name="ot", tag="ot", bufs=2)
                    nc.scalar.add(ot[:], pj[:], pbt[:, co:co + 1])
                    nc.sync.dma_start(out.ap()[co * 128:(co + 1) * 128, :],
                                      ot[:])
                nc.leave_named_scope("proj", sc[0], False)

    nc.compile()
    return nc


def prep_inputs(x, qkv_w, proj_w, proj_b, slopes):
    """Build the 8 per-core input maps (all host-side numpy)."""
    x = np.asarray(x, np.float32)
    qkv_w = np.asarray(qkv_w, np.float32)
    proj_w = np.asarray(proj_w, np.float32)
    proj_b = np.asarray(proj_b, np.float32)
    slopes = np.asarray(slopes, np.float32)

    # RoPE tables (transposed [d, n], d pairwise-interleaved, x2 head copies)
    inv = 1.0 / (10000.0 ** (np.arange(0, D, 2, dtype=np.float32) / D))
    fr = np.arange(N, dtype=np.float32)[:, None] * inv[None, :]   # [N, 32]
    sin_t, cos_t = np.sin(fr), np.cos(fr)
    ct64 = np.empty((64, N), np.float32)
    st64 = np.empty((64, N), np.float32)
    ct64[0::2] = cos_t.T
    ct64[1::2] = cos_t.T
    st64[0::2] = -sin_t.T
    st64[1::2] = sin_t.T
    ctq = (np.vstack([ct64, ct64]) * SCALE).astype(NPBF)
    stq = (np.vstack([st64, st64]) * SCALE).astype(NPBF)
    ctk = np.vstack([ct64, ct64]).astype(NPBF)
    stk = np.vstack([st64, st64]).astype(NPBF)

    pos_p = np.arange(128, dtype=np.float64)[:, None]
    t_off = np.arange(EB_W, dtype=np.float64)[None, :] - EB_OFF
    dmin = np.minimum(pos_p - t_off, 0.0)  # j - i clipped

    in_maps = []
    for core in range(N_CORES):
        b = core // 4
        g = core % 4
        heads = GROUP_HEADS[g]
        rows = []
        for kind in range(2):  # q, k (d-permuted)
            for h in heads:
                base = kind * C + h * D
                rows.extend(base + p for p in D_PERM)
        for h in heads:        # v (natural d order)
            rows.extend(2 * C + h * D + d for d in range(D))
        wT_c = np.ascontiguousarray(qkv_w[rows, :].T)      # [1024, 768]

        eb_c = np.empty((G, 128, EB_W), np.float32)
        for s, h in enumerate(heads):
            eb_c[s] = np.exp(float(slopes[h]) * MAX_BIAS * dmin)

        # pwT rows (ci) ordered as the two AG outputs: half h rows =
        # [rank0 (slot 2h, 2h+1), rank1, ..., rank3] x 64 d each.
        pwT_c = np.empty((C, C), np.float32)
        for s in range(4):
            for rank in range(4):
                hh = GROUP_HEADS[rank][s]
                r = 256 * s + 64 * rank
                pwT_c[r:r + 64, :] = proj_w[:, hh * D:(hh + 1) * D].T
        pb_c = np.ascontiguousarray(proj_b.reshape(8, 128).T)

        # quarter-gather row indices into ag_out viewed as [(r q) w]
        qidx_c = np.empty((128, 2), np.uint32)
        p_ = np.arange(128)
        for hh in range(2):
            rank = 2 * hh + p_ // 64
            qidx_c[:, hh] = 64 * (4 * b + rank) + p_ % 64

        in_maps.append({
            "xT": np.ascontiguousarray(x[b].T).astype(NPBF),
            "wT": wT_c.astype(NPBF),
            "ctq": ctq, "stq": stq, "ctk": ctk, "stk": stk,
            "ebias": eb_c.astype(NPBF),
            "pwT": pwT_c.astype(NPBF),
            "pb": pb_c,
            "qidx": qidx_c,
        })
    return in_maps


_NC = None


def _get_nc():
    global _NC
    if _NC is None:
        _NC = build_program()
    return _NC


def run(inputs, trace=False):
    nc = _get_nc()
    in_maps = prep_inputs(**inputs)
    res = run_bass_kernel_spmd(nc, in_maps, core_ids=list(range(N_CORES)),
                               trace=trace)
    out = np.empty((B, N, C), np.float32)
    for core in range(N_CORES):
        b, g = core // 4, core % 4
        out[b, g * 512:(g + 1) * 512, :] = res.results[core]["out"].T
    return out, res


def kernel(**inputs) -> np.ndarray:
    out, _ = run(inputs, trace=False)
    return out
